# revision 32
# baseline (speedup 1.0000x reference)
"""CANLayer (cell attention) distributed Bass kernel for 8 TRN2 NeuronCores.

Strategy (graph/data parallel by destination cell, per sharding hint):
 - core k owns target nodes [k*LOCAL, (k+1)*LOCAL)
 - the per-node table row id is partition-major (rho = p*NW + w) so the
   phase-1 table store is a contiguous per-partition DMA.
 - one AllGather per set (chunking proved counterproductive: each collective
   pays a ~30us peer-barrier); AG-L fires right after the L table is stored,
   AG-U after the U table, both before any gathers on the in-order gpsimd
   queue (their inputs are ready by then, so they don't stall gather
   dispatch).
 - edge phase per set: edges sorted by (src-core-half, target-window);
   8-tile dma_gather calls read only the 272B payload of each 512B-strided
   table row on 4 SWDGE queues with a ring of gather buffers; per 128-edge
   tile build one-hot S [e,t] via DVE compare vs iota; S^T via ones-row PE
   broadcast + DVE compare; alpha -> lrelu -> exp; scatter =
   matmul(lhsT=S, rhs=[M*e_att | e_att]) accumulated in PSUM per 128-target
   window; per-set batched epilogue normalizes by the denominator columns;
   final relu + single p-major output store.
"""
import sys

if "/opt/trn_rl_repo" not in sys.path:
    sys.path.insert(0, "/opt/trn_rl_repo")

import numpy as np

TRACE = False          # test.py sets True to get exec_time_ns + perfetto
LAST_RESULT = {}       # test.py reads exec_time_ns etc. from here

NCORES = 8
WIN = 128              # targets per PSUM window
NW = 50                # windows per core
SH = NW * WIN          # padded rows per core (6400)
CALL_TILES = 8         # max 128-edge tiles per dma_gather call
RING = 8               # gather ring depth (pool bufs)
COLL_BYPASS = False    # diagnostic: replace AllGather with local shard copy
EW = 68                # f32 cols gathered per edge (272B payload)


def _dma_gather_slim(gp, out_ap, in_ap, idxs_ap, num_idxs, num_idxs_reg,
                     elem_size, elem_step, queue_num=0):
    """nc.gpsimd.dma_gather with the elem_size%256B assert relaxed.

    The SWDGE ISA encodes the row stride in 256B units (elem_step must be a
    256B multiple) but the per-descriptor read size is a plain byte count;
    reading a 272B payload out of 512B-strided rows is legal at the
    descriptor level. Only HBM-source, transpose=False is supported here.
    """
    import concourse.mybir as mybir
    from concourse import ap_utils
    from concourse.bass_primitives import MemorySpace
    from concourse._compat import round_up_to_multiple, exact_div

    gp._assert_queue_num(queue_num)
    assert idxs_ap.dtype == mybir.dt.int16
    assert in_ap.dtype == out_ap.dtype
    assert in_ap.space == MemorySpace.DRAM
    assert idxs_ap.space == MemorySpace.SBUF
    assert out_ap.space == MemorySpace.SBUF
    assert ap_utils.ap_is_contiguous(out_ap.ap[1:])
    assert ap_utils.ap_is_contiguous(idxs_ap.ap[1:])
    assert in_ap.ap[-1][1] == out_ap.ap[-1][1] == elem_size
    assert out_ap.ap[0][1] * out_ap.ap[1][1] == round_up_to_multiple(num_idxs, 128)
    assert in_ap.ap[0][0] == elem_step
    stride_bytes = elem_step * mybir.dt.size(in_ap.dtype)
    stride_bytes_256 = exact_div(stride_bytes, 256)
    assert stride_bytes_256 < 256

    _in_ap = gp.lower_ap_dma(in_ap, for_custom_bir_dma=True)
    _idxs_ap = gp.lower_ap(idxs_ap)
    _out_ap = gp.lower_ap(out_ap)
    return gp.add_instruction(
        mybir.InstDMAGatherAnt(
            name=gp.bass.get_next_instruction_name(),
            ins=[*_in_ap, _idxs_ap,
                 gp.lower_val_access(gp.to_reg(num_idxs_reg))],
            outs=[_out_ap],
            transpose=False,
            num_idxs=num_idxs,
            elem_size=elem_size,
            stride_bytes_256=stride_bytes_256,
            gen_mode=0,
            single_packet=True,
            queue_num=queue_num,
            sbuf_tokens_per_rank=0,
            sbuf_free_dim_per_rank=0,
            sbuf_free_dim_pad_per_rank=0,
            sbuf_byte_offset=0,
        )
    )


def _binpack_windows(degs, local, budget):
    """degs: [n_kinds, local] per-node edge counts. Returns flat slot
    (w*WIN + pos) per node. Greedy first-fit-decreasing."""
    budgets = np.full((NW, degs.shape[0]), budget, np.int64)
    budgets[max(0, NW - 6):] = 6 * WIN
    counts = np.zeros((NW, degs.shape[0]), np.int64)
    nslots = np.zeros(NW, np.int64)
    win_of = np.full(local, -1, np.int64)
    order = np.argsort(-degs.sum(0), kind="stable")
    cap_slots = np.full(NW, local // NW, np.int64)
    cap_slots[:local - (local // NW) * NW] += 1
    for j in order:
        d = degs[:, j]
        over = np.maximum(counts + d[None, :] - budgets, 0).sum(1)
        slack = (budgets - counts - d[None, :]).sum(1)
        cost = over * 100000 - slack
        cost[nslots >= cap_slots] = 1 << 60
        w = int(np.argmin(cost))
        counts[w] += d
        win_of[j] = w
        nslots[w] += 1
    slot = np.empty(local, np.int64)
    used = np.zeros(NW, np.int64)
    for j in range(local):
        w = win_of[j]
        slot[j] = w * WIN + used[w]
        used[w] += 1
    return slot


def _make_perms(lower_tgt, lower_src, upper_tgt, upper_src, local):
    """perm[k][t] = global slot (window-major) of node t."""
    perms = []
    for k in range(NCORES):
        degs = []
        for tg, sr in ((lower_tgt, lower_src), (upper_tgt, upper_src)):
            m = (tg // local) == k
            tl = (tg[m] - k * local).astype(np.int64)
            ch = (sr[m].astype(np.int64) // local) // 4   # src core half
            for h in (0, 1):
                degs.append(np.bincount(tl[ch == h], minlength=local))
        perms.append(_binpack_windows(np.stack(degs), local, 505))
    return perms


def _rho(slot):
    """table row id: partition-major (p*NW + w)."""
    w = slot // WIN
    p = slot % WIN
    return (p * NW + w).astype(np.int64)


# ----------------------------------------------------------------------------
# host-side index preprocessing (pure layout/index manipulation)
# ----------------------------------------------------------------------------

def _preprocess(tgt, src, local, perms):
    """Shard one edge set by destination core; sort by (src-half, window).

    Static structure (caps, groups, calls) is shared across cores (maxed)
    as required for SPMD; per-core arrays carry indices + relative targets.
    """
    per_core = []
    for k in range(NCORES):
        m = (tgt // local) == k
        s = src[m].astype(np.int64)
        tl = perms[k][(tgt[m] - k * local).astype(np.int64)]
        c = (s // local) // 4        # src core half
        w = tl // WIN
        order = np.lexsort((tl, w, c))
        per_core.append((s[order], tl[order], w[order], c[order]))

    caps = np.zeros((2, NW), np.int64)
    for k in range(NCORES):
        s, tl, w, c = per_core[k]
        for cc in range(2):
            cnt = np.bincount(w[c == cc], minlength=NW)
            caps[cc] = np.maximum(caps[cc], (cnt + WIN - 1) // WIN)

    # groups in stream order; calls are CALL_TILES-sized slices of each
    # bucket run (groups may span calls)
    groups = []          # (cc, w, cap, tile_start)
    calls = []           # (cc, tile_start, n_tiles)
    t_idx = 0
    for cc in range(2):
        run_start = t_idx
        for w in range(NW):
            cap = int(caps[cc][w])
            if cap == 0:
                continue
            groups.append((cc, w, cap, t_idx))
            t_idx += cap
        for c0 in range(run_start, t_idx, CALL_TILES):
            calls.append((cc, c0, min(CALL_TILES, t_idx - c0)))
    T = t_idx

    cores = []
    for k in range(NCORES):
        s, tl, w, c = per_core[k]
        slots = T * 128
        src16 = np.zeros(slots, np.int16)          # pad -> idx 0 (valid row)
        trel = np.full(slots, -1.0, np.float32)    # pad -> -1 (no S match)
        for (cc, wg, cap, t0) in groups:
            sel = (c == cc) & (w == wg)
            n = int(sel.sum())
            off = t0 * 128
            ss = s[sel]
            sloc = np.empty(len(ss), np.int64)
            for ks in range(NCORES):
                mm = (ss // local) == ks
                sloc[mm] = perms[ks][ss[mm] % local]
            src16[off:off + n] = (((ss // local) % 4) * SH + _rho(sloc)
                                  ).astype(np.int16)
            trel[off:off + n] = (tl[sel] - wg * WIN).astype(np.float32)
        ii = np.arange(slots)
        idxarr = np.zeros((128, T * 8), np.int16)
        for g8 in range(8):
            idxarr[g8 * 16 + ii % 16, ii // 16] = src16
        trelarr = np.full((128, T), -1.0, np.float32)
        trelarr[ii % 128, ii // 128] = trel
        cores.append({"idx": idxarr, "trel": trelarr})
    # every window must receive at least one scatter flush, else the batched
    # epilogue would read garbage accumulators for it
    assert {w for (_, w, _, _) in groups} == set(range(NW))
    return caps, groups, calls, T, cores


def _block_diag_a(a):  # [H, C] -> [H*C, H] block diagonal (layout only)
    h, c = a.shape
    out = np.zeros((h * c, h), np.float32)
    for i in range(h):
        out[i * c:(i + 1) * c, i] = a[i]
    return out


# ----------------------------------------------------------------------------
# device kernel builder
# ----------------------------------------------------------------------------

def _build(meta):
    import concourse.bass as bass
    import concourse.bacc as bacc
    import concourse.mybir as mybir
    import concourse.tile as tile

    F32 = mybir.dt.float32
    BF16 = mybir.dt.bfloat16
    I16 = mybir.dt.int16
    I32 = mybir.dt.int32
    AL = mybir.AluOpType
    ACTF = mybir.ActivationFunctionType

    eps_skip = meta["eps_skip"]

    nc = bacc.Bacc("TRN2", target_bir_lowering=False, debug=False,
                   num_devices=NCORES, num_swdge_queues=4)

    xT = nc.dram_tensor("xT", [128, SH], BF16, kind="ExternalInput")
    # wall = [Wl | Wl@As_l | Wl@Ad_l | Wu | Wu@As_u | Wu@Ad_u | Wskip]
    Wall = nc.dram_tensor("Wall", [128, 400], BF16, kind="ExternalInput")
    # Aexp[(t,h), t'*128+c] = (t'==t) & (c//32==h): expands alphaT [32,e]
    # to a dense per-channel multiplier [e, nt*128] via one PE matmul
    AexpT = nc.dram_tensor("AexpT", [32, CALL_TILES * 128], BF16,
                           kind="ExternalInput")
    out_ext = nc.dram_tensor("out", [SH, 128], F32, kind="ExternalOutput")

    sets = []
    for z, zn in enumerate("LU"):
        TZ = meta["T"][z]
        sets.append(dict(
            z=z, zn=zn,
            idx=nc.dram_tensor(f"idx{zn}", [128, TZ * 8], I16, kind="ExternalInput"),
            trel=nc.dram_tensor(f"trel{zn}", [128, TZ], F32, kind="ExternalInput"),
            ag_in=nc.dram_tensor(f"agin{zn}", [SH, 128], F32),
            ag_out=nc.dram_tensor(f"agout{zn}", [NCORES * SH, 128], F32,
                                  addr_space="Shared"),
            caps=meta["caps"][z], groups=meta["groups"][z],
            calls=meta["calls"][z], T=TZ,
        ))

    rg = [list(range(NCORES))]

    with tile.TileContext(nc) as tc:
        with (
            tc.tile_pool(name="const", bufs=1) as constp,
            tc.tile_pool(name="p1", bufs=3) as p1,
            tc.tile_pool(name="gat", bufs=RING) as gatp,
            tc.tile_pool(name="work", bufs=6) as workp,
            tc.tile_pool(name="small", bufs=4) as smallp,
            tc.tile_pool(name="winb", bufs=1) as winp,
            tc.tile_pool(name="psA", bufs=2, space="PSUM") as psA,
            tc.tile_pool(name="psT", bufs=2, space="PSUM") as psT,
            tc.tile_pool(name="psE", bufs=2, space="PSUM") as psE,
            tc.tile_pool(name="psW", bufs=2, space="PSUM") as psW,
        ):
            # ---------------- constants ----------------
            # xall + weights first: they gate phase 1 (idx/trel loads are
            # emitted in the edge section so they don't queue ahead)
            xall = winp.tile([128, NW * 128], BF16)
            nc.sync.dma_start(xall[:], xT[:])
            wall = constp.tile([128, 400], BF16)
            nc.sync.dma_start(wall[:], Wall[:])
            aexp = constp.tile([32, CALL_TILES * 128], BF16)
            nc.sync.dma_start(aexp[:], AexpT[:])

            iota_i = constp.tile([128, 128], I32)
            nc.gpsimd.iota(iota_i[:], [[1, 128]], base=0, channel_multiplier=0)
            iota_bf = constp.tile([128, 128], BF16)
            nc.vector.tensor_copy(iota_bf[:], iota_i[:])
            iodiag = constp.tile([128, 128], I32)
            nc.gpsimd.iota(iodiag[:], [[1, 128]], base=0, channel_multiplier=-1)
            ident_bf = constp.tile([128, 128], BF16)
            nc.vector.tensor_single_scalar(ident_bf[:], iodiag[:], 0.0, AL.is_equal)

            # ---------------- persistent buffers ----------------
            out_acc = winp.tile([128, NW, 128], F32)
            tw = winp.tile([128, NW, 8], BF16)
            wacc = winp.tile([128, NW, 132], F32)      # reused across sets

            def emit_ag(st):
                if COLL_BYPASS:
                    nc.sync.dma_start(st["ag_out"][0:SH, :], st["ag_in"][:])
                else:
                    nc.gpsimd.collective_compute(
                        "AllGather", AL.bypass, replica_groups=rg,
                        ins=[st["ag_in"][:].opt()],
                        outs=[st["ag_out"][:].opt()])

            # ---------------- phase 1 ----------------
            # table rows are partition-major: row rho = p*NW + w
            # -> contiguous per-partition store
            def build_table_pair(st, ps, w, wn, z):
                """table rows for windows [w, w+wn) from psum [128, wn*136]"""
                tbl = p1.tile([128, 2 * 128], F32, tag=f"tbl{z}")
                tblb = tbl[:].bitcast(BF16)
                ps3 = ps[:, 0:wn * 136].rearrange("p (w c) -> p w c", c=136)
                tb3 = tblb[:, 0:wn * 256].rearrange("p (w c) -> p w c", c=256)
                tf3 = tbl[:, 0:wn * 128].rearrange("p (w c) -> p w c", c=128)
                # alternate the big xm cast between engines
                if (w // 2) % 2 == 0:
                    nc.scalar.copy(tb3[:, :, 0:128], ps3[:, :, 0:128])
                else:
                    nc.vector.tensor_copy(tb3[:, :, 0:128], ps3[:, :, 0:128])
                nc.vector.tensor_copy(tf3[:, :, 64:68], ps3[:, :, 128:132])
                nc.vector.tensor_copy(tw[:, w:w + wn, z * 4:z * 4 + 4],
                                      ps3[:, :, 132:136])
                dst = st["ag_in"][:].rearrange(
                    "(p w) c -> p w c", p=128)[:, w:w + wn, :]
                nc.sync.dma_start(dst, tf3)

            for z, st in enumerate(sets):
                for w in range(0, NW, 2):
                    wn = min(2, NW - w)
                    ps = psA.tile([128, 280], F32, tag="p1ps")
                    for j in range(wn):
                        nc.tensor.matmul(
                            ps[:, j * 136:j * 136 + 136],
                            lhsT=xall[:, (w + j) * 128:(w + j + 1) * 128],
                            rhs=wall[:, z * 136:z * 136 + 136],
                            start=True, stop=True)
                    if z == 1:
                        psk = psT.tile([128, 512], F32, tag="sTt")
                        for j in range(wn):
                            nc.tensor.matmul(
                                psk[:, j * 128:j * 128 + 128],
                                lhsT=xall[:, (w + j) * 128:(w + j + 1) * 128],
                                rhs=wall[:, 272:400], start=True, stop=True)
                        nc.scalar.activation(
                            out_acc[:, w:w + wn, :],
                            psk[:, 0:wn * 128].rearrange(
                                "p (w c) -> p w c", c=128),
                            ACTF.Copy, scale=eps_skip)
                    build_table_pair(st, ps, w, wn, z)
                emit_ag(st)

            # ---------------- edge phase ----------------
            for z, st in enumerate(sets):
                zn = st["zn"]
                st["idx_sb"] = constp.tile([128, st["T"] * 8], I16,
                                           tag=f"idxsb{zn}", name=f"idxsb{zn}")
                nc.sync.dma_start(st["idx_sb"][:], st["idx"][:])
                st["trel_f"] = constp.tile([128, st["T"]], F32,
                                           tag=f"trelf{zn}", name=f"trelf{zn}")
                nc.sync.dma_start(st["trel_f"][:], st["trel"][:])
                groups, calls = st["groups"], st["calls"]
                trel_f, idx_sb = st["trel_f"], st["idx_sb"]

                # tile index -> (gather ring tile, position-in-call)
                tile_loc = {}
                call_of = {}
                for ci, (cc, t0, nt) in enumerate(calls):
                    g = gatp.tile([128, CALL_TILES * EW], F32, tag="gring")
                    dst = g[:, 0:nt * EW].rearrange("p (t e) -> p t e", e=EW)
                    nidx = nt * 128
                    _dma_gather_slim(
                        nc.gpsimd, dst,
                        st["ag_out"][cc * 4 * SH:(cc + 1) * 4 * SH, 0:EW],
                        idx_sb[:, t0 * 8:t0 * 8 + nt * 8], nidx, nidx, EW,
                        128, queue_num=ci % 4)
                    for j in range(nt):
                        tile_loc[t0 + j] = (g, j)
                        call_of[t0 + j] = ci

                # per-call batched alpha/e_att/scale state
                call_state = {}

                wg_of = {}
                for gi, (cc, wg, cap, t0) in enumerate(groups):
                    for j in range(cap):
                        wg_of[t0 + j] = wg

                def process_call(ci):
                    """S builds, S^T via PE transpose, t-expand, alpha,
                    exp, PE-expanded scale — batched per call."""
                    cc, t0, nt = calls[ci]
                    g = tile_loc[t0][0]
                    # one-hot S [e, t] per tile (tensor_scalar, bf16 4x)
                    Sc = workp.tile([128, CALL_TILES * 128], BF16, tag="S",
                                    name=f"S_{z}_{ci}", bufs=6)
                    for j in range(nt):
                        nc.vector.tensor_scalar(
                            Sc[:, j * 128:(j + 1) * 128], iota_bf[:],
                            trel_f[:, t0 + j:t0 + j + 1], None, AL.is_equal)
                    # S^T via PE transpose (batched into one psum bank)
                    pT = psT.tile([128, CALL_TILES * 128], BF16, tag="sTt",
                                  name=f"pT_{z}_{ci}")
                    for j in range(nt):
                        nc.tensor.transpose(pT[:, j * 128:(j + 1) * 128],
                                            Sc[:, j * 128:(j + 1) * 128],
                                            ident_bf[:])
                    STc = workp.tile([128, CALL_TILES * 128], BF16, tag="STc",
                                     name=f"STc_{z}_{ci}", bufs=6)
                    nc.scalar.copy(STc[:, 0:nt * 128], pT[:, 0:nt * 128])
                    # t-expand per tile (PE), results into one per-call bank
                    pte = psA.tile([128, CALL_TILES * 4], F32, tag="p1ps",
                                   name=f"pte_{z}_{ci}")
                    for j in range(nt):
                        wg = wg_of[t0 + j]
                        nc.tensor.matmul(
                            pte[:, j * 4:j * 4 + 4],
                            lhsT=STc[:, j * 128:(j + 1) * 128],
                            rhs=tw[:, wg, z * 4:z * 4 + 4],
                            start=True, stop=True)
                    # alpha = s + t ; lrelu ; exp (bf16)
                    al = smallp.tile([128, CALL_TILES * 4], F32, tag="al",
                                     name=f"al_{z}_{ci}")
                    alv = al[:, 0:nt * 4]
                    s_ap = g[:, 64:68]
                    s_ap3 = bass.AP(s_ap.tensor, s_ap.offset,
                                    [s_ap.ap[0], [EW, nt], [1, 4]])
                    nc.vector.tensor_tensor(
                        alv.rearrange("p (t f) -> p t f", f=4), s_ap3,
                        pte[:, 0:nt * 4].rearrange("p (t f) -> p t f", f=4),
                        AL.add)
                    nc.vector.scalar_tensor_tensor(alv, alv, 0.01, alv,
                                                   AL.mult, AL.max)
                    alx = smallp.tile([128, CALL_TILES * 4], BF16, tag="alx",
                                      name=f"alx_{z}_{ci}")
                    nc.scalar.activation(alx[:, 0:nt * 4], alv, ACTF.Exp)
                    # alphaT [nt*4, e] -> dense per-channel multiplier E via
                    # one block-diag PE matmul, staged to SBUF by Scalar
                    pAT = psA.tile([128, 128], BF16, tag="p1ps",
                                   name=f"pAT_{z}_{ci}")
                    nc.tensor.transpose(pAT[0:nt * 4, :], alx[:, 0:nt * 4],
                                        ident_bf[:])
                    aT = smallp.tile([32, 128], BF16, tag="aT",
                                     name=f"aT_{z}_{ci}")
                    nc.scalar.copy(aT[0:nt * 4, :], pAT[0:nt * 4, :])
                    Eb = workp.tile([128, CALL_TILES * 128], BF16, tag="Eb",
                                    name=f"Eb_{z}_{ci}", bufs=6)
                    for mm0 in range(0, nt * 128, 512):
                        mm1 = min(mm0 + 512, nt * 128)
                        pE = psE.tile([128, 512], F32, tag="Ee",
                                      name=f"pE_{z}_{ci}_{mm0}")
                        nc.tensor.matmul(pE[:, 0:mm1 - mm0],
                                         lhsT=aT[0:nt * 4, :],
                                         rhs=aexp[0:nt * 4, mm0:mm1],
                                         start=True, stop=True)
                        nc.scalar.copy(Eb[:, mm0:mm1], pE[:, 0:mm1 - mm0])
                    # B = [xm * e_att | e_att]
                    B = workp.tile([128, CALL_TILES, 132], BF16, tag="B",
                                   name=f"B_{z}_{ci}", bufs=6)
                    eb_src = bass.AP(Eb[:].tensor, Eb[:].offset,
                                     [Eb[:].ap[0], [128, nt], [32, 4]])
                    nc.vector.tensor_copy(B[:, 0:nt, 128:132], eb_src)
                    gbf = g[:].bitcast(BF16)
                    mb = bass.AP(gbf.tensor, gbf.offset,
                                 [gbf.ap[0], [2 * EW, nt], [1, 128]])
                    nc.vector.tensor_tensor(
                        B[:, 0:nt, 0:128], mb,
                        Eb[:, 0:nt * 128].rearrange("p (t c) -> p t c", c=128),
                        AL.mult)
                    call_state[ci] = (Sc, B)

                # scatter matmuls in stream order, windows accumulate in PSUM
                flushed = set()
                for gi, (cc, wg, cap, t0) in enumerate(groups):
                    pw = psW.tile([128, 132], F32, tag="pw",
                                  name=f"pw_{z}_{cc}_{wg}")
                    for j in range(cap):
                        ci = call_of[t0 + j]
                        if ci not in call_state:
                            process_call(ci)
                            # retire old call states (ring depth)
                            for old in [k for k in call_state
                                        if k < ci - RING + 1]:
                                del call_state[old]
                        Sc, B = call_state[ci]
                        _, jj = tile_loc[t0 + j]
                        nc.tensor.matmul(pw[:],
                                         lhsT=Sc[:, jj * 128:(jj + 1) * 128],
                                         rhs=B[:, jj, :],
                                         start=(j == 0), stop=(j == cap - 1))
                    if wg not in flushed:
                        nc.scalar.copy(wacc[:, wg, :], pw[:])
                        flushed.add(wg)
                    else:
                        nc.vector.tensor_add(wacc[:, wg, :],
                                             wacc[:, wg, :], pw[:])

                # ---- batched epilogue for this set ----
                # rec = 1/(den+eps); out_acc += num * rec (head-broadcast)
                den = smallp.tile([128, NW * 4], F32, tag="den",
                                  name=f"den_{z}")
                nc.vector.tensor_single_scalar(
                    den[:].rearrange("p (w f) -> p w f", f=4),
                    wacc[:, :, 128:132], 1e-16, AL.add)
                rec = smallp.tile([128, NW * 4], F32, tag="rec",
                                  name=f"rec_{z}")
                nc.vector.reciprocal(rec[:], den[:])
                num4 = wacc[:, :, 0:128].rearrange(
                    "p w (h c) -> p w h c", h=4)
                rec4 = rec[:].rearrange("p (w h o) -> p w h o", h=4, o=1)
                n4, r4 = bass.broadcast_tensor_aps(num4, rec4)
                tmp = winp.tile([128, NW, 128], F32, tag="tmpn",
                                name=f"tmp_{z}")
                nc.vector.tensor_tensor(
                    tmp[:].rearrange("p w (h c) -> p w h c", h=4),
                    n4, r4, AL.mult)
                nc.vector.tensor_add(out_acc[:], out_acc[:], tmp[:])

            # ---- final relu + single p-major store ----
            nc.scalar.activation(out_acc[:], out_acc[:], ACTF.Relu)
            dst = out_ext[:].rearrange("(p w) c -> p w c", p=128)
            nc.sync.dma_start(dst, out_acc[:])

    nc.compile()
    return nc


# ----------------------------------------------------------------------------
# entry point
# ----------------------------------------------------------------------------

def _prepare(x, W_low, a_src_low, a_dst_low, W_up, a_src_up, a_dst_up, W_skip,
             lower_tgt, lower_src, upper_tgt, upper_src):
    n, inch = x.shape
    local = n // NCORES
    assert local == 6250 and inch == 128
    assert 4 * SH <= 32767, "int16 gather index overflow"

    lower_tgt = np.asarray(lower_tgt); lower_src = np.asarray(lower_src)
    upper_tgt = np.asarray(upper_tgt); upper_src = np.asarray(upper_src)
    perms = _make_perms(lower_tgt, lower_src, upper_tgt, upper_src, local)
    capsL, groupsL, callsL, TL, coresL = _preprocess(
        lower_tgt, lower_src, local, perms)
    capsU, groupsU, callsU, TU, coresU = _preprocess(
        upper_tgt, upper_src, local, perms)

    meta = dict(eps_skip=1.0 + 1e-6,
                caps=[capsL, capsU], groups=[groupsL, groupsU],
                calls=[callsL, callsU], T=[TL, TU], perms=perms)

    import ml_dtypes
    W_low = np.asarray(W_low, np.float32)
    W_up = np.asarray(W_up, np.float32)
    W_skip = np.asarray(W_skip, np.float32)
    wall = np.concatenate([
        W_low, W_low @ _block_diag_a(np.asarray(a_src_low)),
        W_low @ _block_diag_a(np.asarray(a_dst_low)),
        W_up, W_up @ _block_diag_a(np.asarray(a_src_up)),
        W_up @ _block_diag_a(np.asarray(a_dst_up)),
        W_skip], axis=1).astype(ml_dtypes.bfloat16)
    aexp = np.zeros((32, CALL_TILES * 128), ml_dtypes.bfloat16)
    for t in range(CALL_TILES):
        for h in range(4):
            aexp[t * 4 + h, t * 128 + h * 32:t * 128 + (h + 1) * 32] = 1.0

    x = np.asarray(x, np.float32)
    in_maps = []
    for k in range(NCORES):
        xk = np.zeros((SH, inch), np.float32)
        xk[perms[k]] = x[k * local:(k + 1) * local]
        in_maps.append({
            "xT": np.ascontiguousarray(xk.T).astype(ml_dtypes.bfloat16),
            "Wall": wall, "AexpT": aexp,
            "idxL": coresL[k]["idx"], "trelL": coresL[k]["trel"],
            "idxU": coresU[k]["idx"], "trelU": coresU[k]["trel"],
        })
    return meta, in_maps, local


def kernel(x, W_low, a_src_low, a_dst_low, W_up, a_src_up, a_dst_up, W_skip,
           lower_tgt, lower_src, upper_tgt, upper_src):
    from concourse.bass_utils import run_bass_kernel_spmd

    meta, in_maps, local = _prepare(
        x, W_low, a_src_low, a_dst_low, W_up, a_src_up, a_dst_up, W_skip,
        lower_tgt, lower_src, upper_tgt, upper_src)
    nc = _build(meta)

    res = run_bass_kernel_spmd(nc, in_maps, list(range(NCORES)), trace=TRACE)
    LAST_RESULT["exec_time_ns"] = res.exec_time_ns
    LAST_RESULT["res"] = res

    n = np.asarray(x).shape[0]
    perms = meta["perms"]
    out = np.empty((n, 128), np.float32)
    for k in range(NCORES):
        ok = np.asarray(res.results[k]["out"])
        # out rows are p-major: row = p*NW + w for slot (w, p)
        sl = perms[k]
        rows = (sl % WIN) * NW + sl // WIN
        out[k * local:(k + 1) * local] = ok[rows]
    return out


# revision 40
# speedup vs baseline: 1.1432x; 1.1432x over previous
"""CANLayer (cell attention) distributed Bass kernel for 8 TRN2 NeuronCores.

Strategy (graph/data parallel by destination cell, per sharding hint):
 - core k owns target nodes [k*LOCAL, (k+1)*LOCAL)
 - the per-node table row id is partition-major (rho = p*NW + w) so the
   phase-1 table store is a contiguous per-partition DMA.
 - one AllGather per set (chunking proved counterproductive: each collective
   pays a ~30us peer-barrier); AG-L fires right after the L table is stored,
   AG-U after the U table, both before any gathers on the in-order gpsimd
   queue (their inputs are ready by then, so they don't stall gather
   dispatch).
 - edge phase per set: edges sorted by (src-core-half, target-window);
   8-tile dma_gather calls read only the 272B payload of each 512B-strided
   table row on 4 SWDGE queues with a ring of gather buffers; per 128-edge
   tile build one-hot S [e,t] via DVE compare vs iota; S^T via ones-row PE
   broadcast + DVE compare; alpha -> lrelu -> exp; scatter =
   matmul(lhsT=S, rhs=[M*e_att | e_att]) accumulated in PSUM per 128-target
   window; per-set batched epilogue normalizes by the denominator columns;
   final relu + single p-major output store.
"""
import sys

if "/opt/trn_rl_repo" not in sys.path:
    sys.path.insert(0, "/opt/trn_rl_repo")

import numpy as np

TRACE = False          # test.py sets True to get exec_time_ns + perfetto
LAST_RESULT = {}       # test.py reads exec_time_ns etc. from here

NCORES = 8
WIN = 128              # targets per PSUM window
NW = 50                # windows per core
SH = NW * WIN          # padded rows per core (6400)
CALL_TILES = 8         # max 128-edge tiles per dma_gather call
RING = 8               # gather ring depth (pool bufs)
COLL_BYPASS = False    # diagnostic: replace AllGather with local shard copy
EW = 68                # f32 cols gathered per edge (272B payload)


def _dma_gather_slim(gp, out_ap, in_ap, idxs_ap, num_idxs, num_idxs_reg,
                     elem_size, elem_step, queue_num=0):
    """nc.gpsimd.dma_gather with the elem_size%256B assert relaxed.

    The SWDGE ISA encodes the row stride in 256B units (elem_step must be a
    256B multiple) but the per-descriptor read size is a plain byte count;
    reading a 272B payload out of 512B-strided rows is legal at the
    descriptor level. Only HBM-source, transpose=False is supported here.
    """
    import concourse.mybir as mybir
    from concourse import ap_utils
    from concourse.bass_primitives import MemorySpace
    from concourse._compat import round_up_to_multiple, exact_div

    gp._assert_queue_num(queue_num)
    assert idxs_ap.dtype == mybir.dt.int16
    assert in_ap.dtype == out_ap.dtype
    assert in_ap.space == MemorySpace.DRAM
    assert idxs_ap.space == MemorySpace.SBUF
    assert out_ap.space == MemorySpace.SBUF
    assert ap_utils.ap_is_contiguous(out_ap.ap[1:])
    assert ap_utils.ap_is_contiguous(idxs_ap.ap[1:])
    assert in_ap.ap[-1][1] == out_ap.ap[-1][1] == elem_size
    assert out_ap.ap[0][1] * out_ap.ap[1][1] == round_up_to_multiple(num_idxs, 128)
    assert in_ap.ap[0][0] == elem_step
    stride_bytes = elem_step * mybir.dt.size(in_ap.dtype)
    stride_bytes_256 = exact_div(stride_bytes, 256)
    assert stride_bytes_256 < 256

    _in_ap = gp.lower_ap_dma(in_ap, for_custom_bir_dma=True)
    _idxs_ap = gp.lower_ap(idxs_ap)
    _out_ap = gp.lower_ap(out_ap)
    return gp.add_instruction(
        mybir.InstDMAGatherAnt(
            name=gp.bass.get_next_instruction_name(),
            ins=[*_in_ap, _idxs_ap,
                 gp.lower_val_access(gp.to_reg(num_idxs_reg))],
            outs=[_out_ap],
            transpose=False,
            num_idxs=num_idxs,
            elem_size=elem_size,
            stride_bytes_256=stride_bytes_256,
            gen_mode=0,
            single_packet=True,
            queue_num=queue_num,
            sbuf_tokens_per_rank=0,
            sbuf_free_dim_per_rank=0,
            sbuf_free_dim_pad_per_rank=0,
            sbuf_byte_offset=0,
        )
    )


def _binpack_windows(degs, local, budget):
    """degs: [n_kinds, local] per-node edge counts. Returns flat slot
    (w*WIN + pos) per node. Greedy first-fit-decreasing."""
    budgets = np.full((NW, degs.shape[0]), budget, np.int64)
    budgets[max(0, NW - 6):] = 6 * WIN
    counts = np.zeros((NW, degs.shape[0]), np.int64)
    nslots = np.zeros(NW, np.int64)
    win_of = np.full(local, -1, np.int64)
    order = np.argsort(-degs.sum(0), kind="stable")
    cap_slots = np.full(NW, local // NW, np.int64)
    cap_slots[:local - (local // NW) * NW] += 1
    for j in order:
        d = degs[:, j]
        over = np.maximum(counts + d[None, :] - budgets, 0).sum(1)
        slack = (budgets - counts - d[None, :]).sum(1)
        cost = over * 100000 - slack
        cost[nslots >= cap_slots] = 1 << 60
        w = int(np.argmin(cost))
        counts[w] += d
        win_of[j] = w
        nslots[w] += 1
    slot = np.empty(local, np.int64)
    used = np.zeros(NW, np.int64)
    for j in range(local):
        w = win_of[j]
        slot[j] = w * WIN + used[w]
        used[w] += 1
    return slot


def _make_perms(lower_tgt, lower_src, upper_tgt, upper_src, local):
    """perm[k][t] = global slot (window-major) of node t."""
    perms = []
    for k in range(NCORES):
        degs = []
        for tg, sr in ((lower_tgt, lower_src), (upper_tgt, upper_src)):
            m = (tg // local) == k
            tl = (tg[m] - k * local).astype(np.int64)
            ch = (sr[m].astype(np.int64) // local) // 4   # src core half
            for h in (0, 1):
                degs.append(np.bincount(tl[ch == h], minlength=local))
        perms.append(_binpack_windows(np.stack(degs), local, 505))
    return perms


def _rho(slot):
    """table row id: partition-major (p*NW + w)."""
    w = slot // WIN
    p = slot % WIN
    return (p * NW + w).astype(np.int64)


# ----------------------------------------------------------------------------
# host-side index preprocessing (pure layout/index manipulation)
# ----------------------------------------------------------------------------

def _preprocess(tgt, src, local, perms):
    """Shard one edge set by destination core; sort by (src-half, window).

    Static structure (caps, groups, calls) is shared across cores (maxed)
    as required for SPMD; per-core arrays carry indices + relative targets.
    """
    per_core = []
    for k in range(NCORES):
        m = (tgt // local) == k
        s = src[m].astype(np.int64)
        tl = perms[k][(tgt[m] - k * local).astype(np.int64)]
        c = (s // local) // 4        # src core half
        w = tl // WIN
        order = np.lexsort((tl, w, c))
        per_core.append((s[order], tl[order], w[order], c[order]))

    caps = np.zeros((2, NW), np.int64)
    for k in range(NCORES):
        s, tl, w, c = per_core[k]
        for cc in range(2):
            cnt = np.bincount(w[c == cc], minlength=NW)
            caps[cc] = np.maximum(caps[cc], (cnt + WIN - 1) // WIN)

    # groups in stream order; calls are CALL_TILES-sized slices of each
    # bucket run (groups may span calls)
    groups = []          # (cc, w, cap, tile_start)
    calls = []           # (cc, tile_start, n_tiles)
    t_idx = 0
    for cc in range(2):
        run_start = t_idx
        for w in range(NW):
            cap = int(caps[cc][w])
            if cap == 0:
                continue
            groups.append((cc, w, cap, t_idx))
            t_idx += cap
        for c0 in range(run_start, t_idx, CALL_TILES):
            calls.append((cc, c0, min(CALL_TILES, t_idx - c0)))
    T = t_idx

    cores = []
    for k in range(NCORES):
        s, tl, w, c = per_core[k]
        slots = T * 128
        src16 = np.zeros(slots, np.int16)          # pad -> idx 0 (valid row)
        trel = np.full(slots, -1.0, np.float32)    # pad -> -1 (no S match)
        for (cc, wg, cap, t0) in groups:
            sel = (c == cc) & (w == wg)
            n = int(sel.sum())
            off = t0 * 128
            ss = s[sel]
            sloc = np.empty(len(ss), np.int64)
            for ks in range(NCORES):
                mm = (ss // local) == ks
                sloc[mm] = perms[ks][ss[mm] % local]
            src16[off:off + n] = (((ss // local) % 4) * SH + _rho(sloc)
                                  ).astype(np.int16)
            trel[off:off + n] = (tl[sel] - wg * WIN).astype(np.float32)
        ii = np.arange(slots)
        idxarr = np.zeros((128, T * 8), np.int16)
        for g8 in range(8):
            idxarr[g8 * 16 + ii % 16, ii // 16] = src16
        trelarr = np.full((128, T), -1.0, np.float32)
        trelarr[ii % 128, ii // 128] = trel
        import ml_dtypes
        cores.append({"idx": idxarr, "trel": trelarr,
                      "trow": trel.reshape(1, T * 128).astype(ml_dtypes.bfloat16)})
    # every window must receive at least one scatter flush, else the batched
    # epilogue would read garbage accumulators for it
    assert {w for (_, w, _, _) in groups} == set(range(NW))
    return caps, groups, calls, T, cores


def _block_diag_a(a):  # [H, C] -> [H*C, H] block diagonal (layout only)
    h, c = a.shape
    out = np.zeros((h * c, h), np.float32)
    for i in range(h):
        out[i * c:(i + 1) * c, i] = a[i]
    return out


# ----------------------------------------------------------------------------
# device kernel builder
# ----------------------------------------------------------------------------

def _build(meta):
    import concourse.bass as bass
    import concourse.bacc as bacc
    import concourse.mybir as mybir
    import concourse.tile as tile

    F32 = mybir.dt.float32
    BF16 = mybir.dt.bfloat16
    I16 = mybir.dt.int16
    I32 = mybir.dt.int32
    AL = mybir.AluOpType
    ACTF = mybir.ActivationFunctionType

    eps_skip = meta["eps_skip"]

    nc = bacc.Bacc("TRN2", target_bir_lowering=False, debug=False,
                   num_devices=NCORES, num_swdge_queues=4)

    xT = nc.dram_tensor("xT", [128, SH], BF16, kind="ExternalInput")
    # wall = [Wl | Wl@As_l | Wl@Ad_l | Wu | Wu@As_u | Wu@Ad_u | Wskip]
    Wall = nc.dram_tensor("Wall", [128, 400], BF16, kind="ExternalInput")
    out_ext = nc.dram_tensor("out", [SH, 128], F32, kind="ExternalOutput")

    sets = []
    for z, zn in enumerate("LU"):
        TZ = meta["T"][z]
        sets.append(dict(
            z=z, zn=zn,
            idx=nc.dram_tensor(f"idx{zn}", [128, TZ * 8], I16, kind="ExternalInput"),
            trel=nc.dram_tensor(f"trel{zn}", [128, TZ], F32, kind="ExternalInput"),
            trow=nc.dram_tensor(f"trow{zn}", [1, TZ * 128], BF16, kind="ExternalInput"),
            ag_in=nc.dram_tensor(f"agin{zn}", [SH, 128], F32),
            ag_out=nc.dram_tensor(f"agout{zn}", [NCORES * SH, 128], F32,
                                  addr_space="Shared"),
            caps=meta["caps"][z], groups=meta["groups"][z],
            calls=meta["calls"][z], T=TZ,
        ))

    rg = [list(range(NCORES))]

    with tile.TileContext(nc) as tc:
        with (
            tc.tile_pool(name="const", bufs=1) as constp,
            tc.tile_pool(name="p1", bufs=3) as p1,
            tc.tile_pool(name="gat", bufs=RING) as gatp,
            tc.tile_pool(name="work", bufs=6) as workp,
            tc.tile_pool(name="small", bufs=4) as smallp,
            tc.tile_pool(name="winb", bufs=1) as winp,
            tc.tile_pool(name="psA", bufs=2, space="PSUM") as psA,
            tc.tile_pool(name="psE", bufs=4, space="PSUM") as psE,
            tc.tile_pool(name="psW", bufs=2, space="PSUM") as psW,
        ):
            # ---------------- constants ----------------
            # xall + weights first: they gate phase 1 (idx/trel loads are
            # emitted in the edge section so they don't queue ahead)
            xall = winp.tile([128, NW * 128], BF16)
            nc.sync.dma_start(xall[:], xT[:])
            wall = constp.tile([128, 400], BF16)
            nc.sync.dma_start(wall[:], Wall[:])

            iota_i = constp.tile([128, 128], I32)
            nc.gpsimd.iota(iota_i[:], [[1, 128]], base=0, channel_multiplier=0)
            iota_bf = constp.tile([128, 128], BF16)
            nc.vector.tensor_copy(iota_bf[:], iota_i[:])
            ones_row = constp.tile([1, 128], BF16)
            nc.vector.memset(ones_row[:], 1.0)
            iota_col = constp.tile([128, 1], F32)
            nc.gpsimd.iota(iota_col[:].bitcast(I32), [[1, 1]], base=0,
                           channel_multiplier=1)
            nc.vector.tensor_copy(iota_col[:], iota_col[:].bitcast(I32))

            # ---------------- persistent buffers ----------------
            out_acc = winp.tile([128, NW, 128], F32)
            tw = winp.tile([128, NW, 8], BF16)
            wacc = winp.tile([128, NW, 132], F32)      # reused across sets

            def emit_ag(st):
                if COLL_BYPASS:
                    nc.sync.dma_start(st["ag_out"][0:SH, :], st["ag_in"][:])
                else:
                    nc.gpsimd.collective_compute(
                        "AllGather", AL.bypass, replica_groups=rg,
                        ins=[st["ag_in"][:].opt()],
                        outs=[st["ag_out"][:].opt()])

            # ---------------- phase 1 ----------------
            # table rows are partition-major: row rho = p*NW + w
            # -> contiguous per-partition store
            def build_table_pair(st, ps, w, wn, z):
                """table rows for windows [w, w+wn) from psum [128, wn*136]"""
                tbl = p1.tile([128, 2 * 128], F32, tag=f"tbl{z}")
                tblb = tbl[:].bitcast(BF16)
                ps3 = ps[:, 0:wn * 136].rearrange("p (w c) -> p w c", c=136)
                tb3 = tblb[:, 0:wn * 256].rearrange("p (w c) -> p w c", c=256)
                tf3 = tbl[:, 0:wn * 128].rearrange("p (w c) -> p w c", c=128)
                # alternate the big xm cast between engines
                if (w // 2) % 2 == 0:
                    nc.scalar.copy(tb3[:, :, 0:128], ps3[:, :, 0:128])
                else:
                    nc.vector.tensor_copy(tb3[:, :, 0:128], ps3[:, :, 0:128])
                nc.vector.tensor_copy(tf3[:, :, 64:68], ps3[:, :, 128:132])
                nc.vector.tensor_copy(tw[:, w:w + wn, z * 4:z * 4 + 4],
                                      ps3[:, :, 132:136])
                dst = st["ag_in"][:].rearrange(
                    "(p w) c -> p w c", p=128)[:, w:w + wn, :]
                nc.sync.dma_start(dst, tf3)

            for z, st in enumerate(sets):
                for w in range(0, NW, 2):
                    wn = min(2, NW - w)
                    ps = psA.tile([128, 280], F32, tag="p1ps")
                    for j in range(wn):
                        nc.tensor.matmul(
                            ps[:, j * 136:j * 136 + 136],
                            lhsT=xall[:, (w + j) * 128:(w + j + 1) * 128],
                            rhs=wall[:, z * 136:z * 136 + 136],
                            start=True, stop=True)
                    if z == 1:
                        psk = psE.tile([128, 512], F32, tag="Ee")
                        for j in range(wn):
                            nc.tensor.matmul(
                                psk[:, j * 128:j * 128 + 128],
                                lhsT=xall[:, (w + j) * 128:(w + j + 1) * 128],
                                rhs=wall[:, 272:400], start=True, stop=True)
                        nc.scalar.activation(
                            out_acc[:, w:w + wn, :],
                            psk[:, 0:wn * 128].rearrange(
                                "p (w c) -> p w c", c=128),
                            ACTF.Copy, scale=eps_skip)
                    build_table_pair(st, ps, w, wn, z)
                emit_ag(st)

            # ---------------- edge phase ----------------
            for z, st in enumerate(sets):
                zn = st["zn"]
                st["idx_sb"] = constp.tile([128, st["T"] * 8], I16,
                                           tag=f"idxsb{zn}", name=f"idxsb{zn}")
                nc.sync.dma_start(st["idx_sb"][:], st["idx"][:])
                st["trel_f"] = constp.tile([128, st["T"]], F32,
                                           tag=f"trelf{zn}", name=f"trelf{zn}")
                nc.sync.dma_start(st["trel_f"][:], st["trel"][:])
                groups, calls = st["groups"], st["calls"]
                trel_f, idx_sb = st["trel_f"], st["idx_sb"]

                # tile index -> (gather ring tile, position-in-call)
                tile_loc = {}
                call_of = {}
                for ci, (cc, t0, nt) in enumerate(calls):
                    g = gatp.tile([128, CALL_TILES * EW], F32, tag="gring")
                    dst = g[:, 0:nt * EW].rearrange("p (t e) -> p t e", e=EW)
                    nidx = nt * 128
                    _dma_gather_slim(
                        nc.gpsimd, dst,
                        st["ag_out"][cc * 4 * SH:(cc + 1) * 4 * SH, 0:EW],
                        idx_sb[:, t0 * 8:t0 * 8 + nt * 8], nidx, nidx, EW,
                        128, queue_num=ci % 4)
                    for j in range(nt):
                        tile_loc[t0 + j] = (g, j)
                        call_of[t0 + j] = ci

                # per-call batched alpha/e_att/scale state
                call_state = {}

                wg_of = {}
                for gi, (cc, wg, cap, t0) in enumerate(groups):
                    for j in range(cap):
                        wg_of[t0 + j] = wg

                def process_call(ci):
                    """S compare, S^T (PE broadcast + 4x compare), t-expand,
                    alpha, exp, scale — batched per call. The S^T/t-expand
                    prep depends only on static inputs, so it runs ahead of
                    the gather-dependent alpha chain."""
                    cc, t0, nt = calls[ci]
                    g = tile_loc[t0][0]
                    Sc = workp.tile([128, CALL_TILES * 128], BF16, tag="S",
                                    name=f"S_{z}_{ci}", bufs=6)
                    iota3 = iota_bf[:].rearrange("p (o e) -> p o e", o=1)
                    trel3 = trel_f[:, t0:t0 + nt].rearrange(
                        "p (t o) -> p t o", o=1)
                    i3, t3 = bass.broadcast_tensor_aps(iota3, trel3)
                    nc.vector.tensor_tensor(
                        Sc[:, 0:nt * 128].rearrange("p (t e) -> p t e", e=128),
                        i3, t3, AL.is_equal)
                    # t-value rows broadcast to all partitions via PE, staged
                    # to SBUF by Scalar so the compare runs at 4x on DVE
                    stg = smallp.tile([1, CALL_TILES * 128], BF16, tag="trowstg",
                                      name=f"stg_{z}_{ci}")
                    nc.sync.dma_start(
                        stg[:, 0:nt * 128],
                        st["trow"][0:1, t0 * 128:(t0 + nt) * 128])
                    pbc_sb = workp.tile([128, CALL_TILES * 128], BF16,
                                        tag="pbcs", name=f"pbcs_{z}_{ci}",
                                        bufs=6)
                    for mm0 in range(0, nt * 128, 512):
                        mm1 = min(mm0 + 512, nt * 128)
                        pbc = psE.tile([128, 512], F32, tag="Ee",
                                       name=f"pbc_{z}_{ci}_{mm0}")
                        nc.tensor.matmul(pbc[:, 0:mm1 - mm0],
                                         lhsT=ones_row[:],
                                         rhs=stg[0:1, mm0:mm1],
                                         start=True, stop=True)
                        nc.scalar.copy(pbc_sb[:, mm0:mm1],
                                       pbc[:, 0:mm1 - mm0])
                    STc = workp.tile([128, CALL_TILES * 128], BF16, tag="STc",
                                     name=f"STc_{z}_{ci}", bufs=6)
                    nc.vector.tensor_scalar(STc[:, 0:nt * 128],
                                            pbc_sb[:, 0:nt * 128],
                                            iota_col[:], None, AL.is_equal)
                    # t-expand per tile (PE), results into one per-call bank
                    pte = psA.tile([128, CALL_TILES * 4], F32, tag="p1ps",
                                   name=f"pte_{z}_{ci}")
                    for j in range(nt):
                        wg = wg_of[t0 + j]
                        nc.tensor.matmul(
                            pte[:, j * 4:j * 4 + 4],
                            lhsT=STc[:, j * 128:(j + 1) * 128],
                            rhs=tw[:, wg, z * 4:z * 4 + 4],
                            start=True, stop=True)
                    # alpha = s + t ; lrelu ; exp -> B ; scale
                    al = smallp.tile([128, CALL_TILES * 4], F32, tag="al",
                                     name=f"al_{z}_{ci}")
                    alv = al[:, 0:nt * 4]
                    s_ap = g[:, 64:68]
                    s_ap3 = bass.AP(s_ap.tensor, s_ap.offset,
                                    [s_ap.ap[0], [EW, nt], [1, 4]])
                    nc.vector.tensor_tensor(
                        alv.rearrange("p (t f) -> p t f", f=4), s_ap3,
                        pte[:, 0:nt * 4].rearrange("p (t f) -> p t f", f=4),
                        AL.add)
                    nc.vector.scalar_tensor_tensor(alv, alv, 0.01, alv,
                                                   AL.mult, AL.max)
                    B = workp.tile([128, CALL_TILES, 132], BF16, tag="B",
                                   name=f"B_{z}_{ci}", bufs=6)
                    nc.scalar.activation(
                        B[:, 0:nt, 128:132],
                        alv.rearrange("p (t f) -> p t f", f=4), ACTF.Exp)
                    gbf = g[:].bitcast(BF16)
                    mb = bass.AP(gbf.tensor, gbf.offset,
                                 [gbf.ap[0], [2 * EW, nt], [32, 4], [1, 32]])
                    b_sl = B[:, 0:nt, 128:132]
                    eb = bass.AP(b_sl.tensor, b_sl.offset,
                                 [*b_sl.ap, [0, 32]])
                    nc.vector.tensor_tensor(
                        B[:, 0:nt, 0:128].rearrange(
                            "p t (h c) -> p t h c", h=4), mb, eb, AL.mult)
                    call_state[ci] = (Sc, B)

                # scatter matmuls in stream order, windows accumulate in PSUM
                flushed = set()
                for gi, (cc, wg, cap, t0) in enumerate(groups):
                    pw = psW.tile([128, 132], F32, tag="pw",
                                  name=f"pw_{z}_{cc}_{wg}")
                    for j in range(cap):
                        ci = call_of[t0 + j]
                        if ci not in call_state:
                            process_call(ci)
                            # retire old call states (ring depth)
                            for old in [k for k in call_state
                                        if k < ci - RING + 1]:
                                del call_state[old]
                        Sc, B = call_state[ci]
                        _, jj = tile_loc[t0 + j]
                        nc.tensor.matmul(pw[:],
                                         lhsT=Sc[:, jj * 128:(jj + 1) * 128],
                                         rhs=B[:, jj, :],
                                         start=(j == 0), stop=(j == cap - 1))
                    if wg not in flushed:
                        nc.scalar.copy(wacc[:, wg, :], pw[:])
                        flushed.add(wg)
                    else:
                        nc.vector.tensor_add(wacc[:, wg, :],
                                             wacc[:, wg, :], pw[:])

                # ---- batched epilogue for this set ----
                # rec = 1/(den+eps); out_acc += num * rec (head-broadcast)
                den = smallp.tile([128, NW * 4], F32, tag="den",
                                  name=f"den_{z}")
                nc.vector.tensor_single_scalar(
                    den[:].rearrange("p (w f) -> p w f", f=4),
                    wacc[:, :, 128:132], 1e-16, AL.add)
                rec = smallp.tile([128, NW * 4], F32, tag="rec",
                                  name=f"rec_{z}")
                nc.vector.reciprocal(rec[:], den[:])
                num4 = wacc[:, :, 0:128].rearrange(
                    "p w (h c) -> p w h c", h=4)
                rec4 = rec[:].rearrange("p (w h o) -> p w h o", h=4, o=1)
                n4, r4 = bass.broadcast_tensor_aps(num4, rec4)
                tmp = winp.tile([128, NW, 128], F32, tag="tmpn",
                                name=f"tmp_{z}")
                nc.vector.tensor_tensor(
                    tmp[:].rearrange("p w (h c) -> p w h c", h=4),
                    n4, r4, AL.mult)
                nc.vector.tensor_add(out_acc[:], out_acc[:], tmp[:])

            # ---- final relu + single p-major store ----
            nc.scalar.activation(out_acc[:], out_acc[:], ACTF.Relu)
            dst = out_ext[:].rearrange("(p w) c -> p w c", p=128)
            nc.sync.dma_start(dst, out_acc[:])

    nc.compile()
    return nc


# ----------------------------------------------------------------------------
# entry point
# ----------------------------------------------------------------------------

def _prepare(x, W_low, a_src_low, a_dst_low, W_up, a_src_up, a_dst_up, W_skip,
             lower_tgt, lower_src, upper_tgt, upper_src):
    n, inch = x.shape
    local = n // NCORES
    assert local == 6250 and inch == 128
    assert 4 * SH <= 32767, "int16 gather index overflow"

    lower_tgt = np.asarray(lower_tgt); lower_src = np.asarray(lower_src)
    upper_tgt = np.asarray(upper_tgt); upper_src = np.asarray(upper_src)
    perms = _make_perms(lower_tgt, lower_src, upper_tgt, upper_src, local)
    capsL, groupsL, callsL, TL, coresL = _preprocess(
        lower_tgt, lower_src, local, perms)
    capsU, groupsU, callsU, TU, coresU = _preprocess(
        upper_tgt, upper_src, local, perms)

    meta = dict(eps_skip=1.0 + 1e-6,
                caps=[capsL, capsU], groups=[groupsL, groupsU],
                calls=[callsL, callsU], T=[TL, TU], perms=perms)

    import ml_dtypes
    W_low = np.asarray(W_low, np.float32)
    W_up = np.asarray(W_up, np.float32)
    W_skip = np.asarray(W_skip, np.float32)
    wall = np.concatenate([
        W_low, W_low @ _block_diag_a(np.asarray(a_src_low)),
        W_low @ _block_diag_a(np.asarray(a_dst_low)),
        W_up, W_up @ _block_diag_a(np.asarray(a_src_up)),
        W_up @ _block_diag_a(np.asarray(a_dst_up)),
        W_skip], axis=1).astype(ml_dtypes.bfloat16)

    x = np.asarray(x, np.float32)
    in_maps = []
    for k in range(NCORES):
        xk = np.zeros((SH, inch), np.float32)
        xk[perms[k]] = x[k * local:(k + 1) * local]
        in_maps.append({
            "xT": np.ascontiguousarray(xk.T).astype(ml_dtypes.bfloat16),
            "Wall": wall,
            "idxL": coresL[k]["idx"], "trelL": coresL[k]["trel"],
            "trowL": coresL[k]["trow"],
            "idxU": coresU[k]["idx"], "trelU": coresU[k]["trel"],
            "trowU": coresU[k]["trow"],
        })
    return meta, in_maps, local


def kernel(x, W_low, a_src_low, a_dst_low, W_up, a_src_up, a_dst_up, W_skip,
           lower_tgt, lower_src, upper_tgt, upper_src):
    from concourse.bass_utils import run_bass_kernel_spmd

    meta, in_maps, local = _prepare(
        x, W_low, a_src_low, a_dst_low, W_up, a_src_up, a_dst_up, W_skip,
        lower_tgt, lower_src, upper_tgt, upper_src)
    nc = _build(meta)

    res = run_bass_kernel_spmd(nc, in_maps, list(range(NCORES)), trace=TRACE)
    LAST_RESULT["exec_time_ns"] = res.exec_time_ns
    LAST_RESULT["res"] = res

    n = np.asarray(x).shape[0]
    perms = meta["perms"]
    out = np.empty((n, 128), np.float32)
    for k in range(NCORES):
        ok = np.asarray(res.results[k]["out"])
        # out rows are p-major: row = p*NW + w for slot (w, p)
        sl = perms[k]
        rows = (sl % WIN) * NW + sl // WIN
        out[k * local:(k + 1) * local] = ok[rows]
    return out


# revision 49
# speedup vs baseline: 1.3116x; 1.1473x over previous
"""CANLayer (cell attention) distributed Bass kernel for 8 TRN2 NeuronCores.

Strategy (graph/data parallel by destination cell, per sharding hint):
 - core k owns target nodes [k*LOCAL, (k+1)*LOCAL)
 - the per-node table row id is partition-major (rho = p*NW + w) so the
   phase-1 table store is a contiguous per-partition DMA.
 - one AllGather per set (chunking proved counterproductive: each collective
   pays a ~30us peer-barrier); AG-L fires right after the L table is stored,
   AG-U after the U table, both before any gathers on the in-order gpsimd
   queue (their inputs are ready by then, so they don't stall gather
   dispatch).
 - edge phase per set: edges sorted by (src-core-half, target-window);
   8-tile dma_gather calls read only the 272B payload of each 512B-strided
   table row on 4 SWDGE queues with a ring of gather buffers; per 128-edge
   tile build one-hot S [e,t] via DVE compare vs iota; S^T via ones-row PE
   broadcast + DVE compare; alpha -> lrelu -> exp; scatter =
   matmul(lhsT=S, rhs=[M*e_att | e_att]) accumulated in PSUM per 128-target
   window; per-set batched epilogue normalizes by the denominator columns;
   final relu + single p-major output store.
"""
import sys

if "/opt/trn_rl_repo" not in sys.path:
    sys.path.insert(0, "/opt/trn_rl_repo")

import numpy as np

TRACE = False          # test.py sets True to get exec_time_ns + perfetto
LAST_RESULT = {}       # test.py reads exec_time_ns etc. from here

NCORES = 8
WIN = 128              # targets per PSUM window
NW = 50                # windows per core
SH = NW * WIN          # padded rows per core (6400)
CALL_TILES = 8         # max 128-edge tiles per dma_gather call
RING = 8               # gather ring depth (pool bufs)
COLL_BYPASS = False    # diagnostic: replace AllGather with local shard copy
EW = 68                # f32 cols gathered per edge (272B payload)
EB_DENSE = True        # expand e_att densely on Scalar for a 2x DVE multiply


def _dma_gather_slim(gp, out_ap, in_ap, idxs_ap, num_idxs, num_idxs_reg,
                     elem_size, elem_step, queue_num=0):
    """nc.gpsimd.dma_gather with the elem_size%256B assert relaxed.

    The SWDGE ISA encodes the row stride in 256B units (elem_step must be a
    256B multiple) but the per-descriptor read size is a plain byte count;
    reading a 272B payload out of 512B-strided rows is legal at the
    descriptor level. Only HBM-source, transpose=False is supported here.
    """
    import concourse.mybir as mybir
    from concourse import ap_utils
    from concourse.bass_primitives import MemorySpace
    from concourse._compat import round_up_to_multiple, exact_div

    gp._assert_queue_num(queue_num)
    assert idxs_ap.dtype == mybir.dt.int16
    assert in_ap.dtype == out_ap.dtype
    assert in_ap.space == MemorySpace.DRAM
    assert idxs_ap.space == MemorySpace.SBUF
    assert out_ap.space == MemorySpace.SBUF
    assert ap_utils.ap_is_contiguous(out_ap.ap[1:])
    assert ap_utils.ap_is_contiguous(idxs_ap.ap[1:])
    assert in_ap.ap[-1][1] == out_ap.ap[-1][1] == elem_size
    assert out_ap.ap[0][1] * out_ap.ap[1][1] == round_up_to_multiple(num_idxs, 128)
    assert in_ap.ap[0][0] == elem_step
    stride_bytes = elem_step * mybir.dt.size(in_ap.dtype)
    stride_bytes_256 = exact_div(stride_bytes, 256)
    assert stride_bytes_256 < 256

    _in_ap = gp.lower_ap_dma(in_ap, for_custom_bir_dma=True)
    _idxs_ap = gp.lower_ap(idxs_ap)
    _out_ap = gp.lower_ap(out_ap)
    return gp.add_instruction(
        mybir.InstDMAGatherAnt(
            name=gp.bass.get_next_instruction_name(),
            ins=[*_in_ap, _idxs_ap,
                 gp.lower_val_access(gp.to_reg(num_idxs_reg))],
            outs=[_out_ap],
            transpose=False,
            num_idxs=num_idxs,
            elem_size=elem_size,
            stride_bytes_256=stride_bytes_256,
            gen_mode=0,
            single_packet=True,
            queue_num=queue_num,
            sbuf_tokens_per_rank=0,
            sbuf_free_dim_per_rank=0,
            sbuf_free_dim_pad_per_rank=0,
            sbuf_byte_offset=0,
        )
    )


def _binpack_windows(degs, local, budget):
    """degs: [n_kinds, local] per-node edge counts. Returns flat slot
    (w*WIN + pos) per node. Greedy first-fit-decreasing."""
    budgets = np.full((NW, degs.shape[0]), budget, np.int64)
    budgets[max(0, NW - 6):] = 6 * WIN
    counts = np.zeros((NW, degs.shape[0]), np.int64)
    nslots = np.zeros(NW, np.int64)
    win_of = np.full(local, -1, np.int64)
    order = np.argsort(-degs.sum(0), kind="stable")
    cap_slots = np.full(NW, local // NW, np.int64)
    cap_slots[:local - (local // NW) * NW] += 1
    for j in order:
        d = degs[:, j]
        over = np.maximum(counts + d[None, :] - budgets, 0).sum(1)
        slack = (budgets - counts - d[None, :]).sum(1)
        cost = over * 100000 - slack
        cost[nslots >= cap_slots] = 1 << 60
        w = int(np.argmin(cost))
        counts[w] += d
        win_of[j] = w
        nslots[w] += 1
    slot = np.empty(local, np.int64)
    used = np.zeros(NW, np.int64)
    for j in range(local):
        w = win_of[j]
        slot[j] = w * WIN + used[w]
        used[w] += 1
    return slot


def _make_perms(lower_tgt, lower_src, upper_tgt, upper_src, local):
    """perm[k][t] = global slot (window-major) of node t."""
    perms = []
    for k in range(NCORES):
        degs = []
        for tg, sr in ((lower_tgt, lower_src), (upper_tgt, upper_src)):
            m = (tg // local) == k
            tl = (tg[m] - k * local).astype(np.int64)
            ch = (sr[m].astype(np.int64) // local) // 4   # src core half
            for h in (0, 1):
                degs.append(np.bincount(tl[ch == h], minlength=local))
        perms.append(_binpack_windows(np.stack(degs), local, 505))
    return perms


def _rho(slot):
    """table row id: partition-major (p*NW + w)."""
    w = slot // WIN
    p = slot % WIN
    return (p * NW + w).astype(np.int64)


# ----------------------------------------------------------------------------
# host-side index preprocessing (pure layout/index manipulation)
# ----------------------------------------------------------------------------

def _preprocess(tgt, src, local, perms):
    """Shard one edge set by destination core; sort by (src-half, window).

    Static structure (caps, groups, calls) is shared across cores (maxed)
    as required for SPMD; per-core arrays carry indices + relative targets.
    """
    per_core = []
    for k in range(NCORES):
        m = (tgt // local) == k
        s = src[m].astype(np.int64)
        tl = perms[k][(tgt[m] - k * local).astype(np.int64)]
        c = (s // local) // 4        # src core half
        w = tl // WIN
        order = np.lexsort((tl, w, c))
        per_core.append((s[order], tl[order], w[order], c[order]))

    caps = np.zeros((2, NW), np.int64)
    for k in range(NCORES):
        s, tl, w, c = per_core[k]
        for cc in range(2):
            cnt = np.bincount(w[c == cc], minlength=NW)
            caps[cc] = np.maximum(caps[cc], (cnt + WIN - 1) // WIN)

    # groups in stream order; calls are CALL_TILES-sized slices of each
    # bucket run (groups may span calls)
    groups = []          # (cc, w, cap, tile_start)
    calls = []           # (cc, tile_start, n_tiles)
    t_idx = 0
    for cc in range(2):
        run_start = t_idx
        for w in range(NW):
            cap = int(caps[cc][w])
            if cap == 0:
                continue
            groups.append((cc, w, cap, t_idx))
            t_idx += cap
        for c0 in range(run_start, t_idx, CALL_TILES):
            calls.append((cc, c0, min(CALL_TILES, t_idx - c0)))
    T = t_idx

    cores = []
    for k in range(NCORES):
        s, tl, w, c = per_core[k]
        slots = T * 128
        src16 = np.zeros(slots, np.int16)          # pad -> idx 0 (valid row)
        trel = np.full(slots, -1.0, np.float32)    # pad -> -1 (no S match)
        for (cc, wg, cap, t0) in groups:
            sel = (c == cc) & (w == wg)
            n = int(sel.sum())
            off = t0 * 128
            ss = s[sel]
            sloc = np.empty(len(ss), np.int64)
            for ks in range(NCORES):
                mm = (ss // local) == ks
                sloc[mm] = perms[ks][ss[mm] % local]
            src16[off:off + n] = (((ss // local) % 4) * SH + _rho(sloc)
                                  ).astype(np.int16)
            trel[off:off + n] = (tl[sel] - wg * WIN).astype(np.float32)
        ii = np.arange(slots)
        idxarr = np.zeros((128, T * 8), np.int16)
        for g8 in range(8):
            idxarr[g8 * 16 + ii % 16, ii // 16] = src16
        trelarr = np.full((128, T), -1.0, np.float32)
        trelarr[ii % 128, ii // 128] = trel
        import ml_dtypes
        cores.append({"idx": idxarr, "trel": trelarr,
                      "trow": trel.reshape(1, T * 128).astype(
                          ml_dtypes.bfloat16)})
    # every window must receive at least one scatter flush, else the batched
    # epilogue would read garbage accumulators for it
    assert {w for (_, w, _, _) in groups} == set(range(NW))
    return caps, groups, calls, T, cores


def _block_diag_a(a):  # [H, C] -> [H*C, H] block diagonal (layout only)
    h, c = a.shape
    out = np.zeros((h * c, h), np.float32)
    for i in range(h):
        out[i * c:(i + 1) * c, i] = a[i]
    return out


# ----------------------------------------------------------------------------
# device kernel builder
# ----------------------------------------------------------------------------

def _build(meta):
    import concourse.bass as bass
    import concourse.bacc as bacc
    import concourse.mybir as mybir
    import concourse.tile as tile

    F32 = mybir.dt.float32
    BF16 = mybir.dt.bfloat16
    I16 = mybir.dt.int16
    I32 = mybir.dt.int32
    AL = mybir.AluOpType
    ACTF = mybir.ActivationFunctionType

    eps_skip = meta["eps_skip"]

    nc = bacc.Bacc("TRN2", target_bir_lowering=False, debug=False,
                   num_devices=NCORES, num_swdge_queues=4)

    xT = nc.dram_tensor("xT", [128, SH], BF16, kind="ExternalInput")
    # wall = [Wl | Wl@As_l | Wl@Ad_l | Wu | Wu@As_u | Wu@Ad_u | Wskip]
    Wall = nc.dram_tensor("Wall", [128, 400], BF16, kind="ExternalInput")
    out_ext = nc.dram_tensor("out", [SH, 128], F32, kind="ExternalOutput")

    sets = []
    for z, zn in enumerate("LU"):
        TZ = meta["T"][z]
        sets.append(dict(
            z=z, zn=zn,
            idx=nc.dram_tensor(f"idx{zn}", [128, TZ * 8], I16, kind="ExternalInput"),
            trel=nc.dram_tensor(f"trel{zn}", [128, TZ], F32, kind="ExternalInput"),
            trow=nc.dram_tensor(f"trow{zn}", [1, TZ * 128], BF16, kind="ExternalInput"),
            ag_in=nc.dram_tensor(f"agin{zn}", [SH, 128], F32),
            ag_out=nc.dram_tensor(f"agout{zn}", [NCORES * SH, 128], F32,
                                  addr_space="Shared"),
            caps=meta["caps"][z], groups=meta["groups"][z],
            calls=meta["calls"][z], T=TZ,
        ))

    rg = [list(range(NCORES))]

    with tile.TileContext(nc) as tc:
        with (
            tc.tile_pool(name="const", bufs=1) as constp,
            tc.tile_pool(name="p1", bufs=3) as p1,
            tc.tile_pool(name="gat", bufs=RING) as gatp,
            tc.tile_pool(name="work", bufs=6) as workp,
            tc.tile_pool(name="small", bufs=4) as smallp,
            tc.tile_pool(name="winb", bufs=1) as winp,
            tc.tile_pool(name="psA", bufs=2, space="PSUM") as psA,
            tc.tile_pool(name="psT", bufs=4, space="PSUM") as psT,
            tc.tile_pool(name="psW", bufs=2, space="PSUM") as psW,
        ):
            # ---------------- constants ----------------
            # xall + weights first: they gate phase 1 (idx/trel loads are
            # emitted in the edge section so they don't queue ahead)
            xall = winp.tile([128, NW * 128], BF16)
            nc.sync.dma_start(xall[:], xT[:])
            wall = constp.tile([128, 400], BF16)
            nc.sync.dma_start(wall[:], Wall[:])

            iota_i = constp.tile([128, 128], I32)
            nc.gpsimd.iota(iota_i[:], [[1, 128]], base=0, channel_multiplier=0)
            iota_bf = constp.tile([128, 128], BF16)
            nc.vector.tensor_copy(iota_bf[:], iota_i[:])
            ones_row = constp.tile([1, 128], BF16)
            nc.vector.memset(ones_row[:], 1.0)
            iota_col = constp.tile([128, 1], F32)
            nc.gpsimd.iota(iota_col[:].bitcast(I32), [[1, 1]], base=0,
                           channel_multiplier=1)
            nc.vector.tensor_copy(iota_col[:], iota_col[:].bitcast(I32))

            # ---------------- persistent buffers ----------------
            out_acc = winp.tile([128, NW, 128], F32)
            tw = winp.tile([128, NW, 8], BF16)
            wacc = winp.tile([128, NW, 132], F32)      # reused across sets

            def emit_ag(st):
                if COLL_BYPASS:
                    nc.sync.dma_start(st["ag_out"][0:SH, :], st["ag_in"][:])
                else:
                    nc.gpsimd.collective_compute(
                        "AllGather", AL.bypass, replica_groups=rg,
                        ins=[st["ag_in"][:].opt()],
                        outs=[st["ag_out"][:].opt()])

            # ---------------- phase 1 ----------------
            # table rows are partition-major: row rho = p*NW + w
            # -> contiguous per-partition store
            def build_table_pair(st, ps, w, wn, z):
                """table rows for windows [w, w+wn) from psum [128, wn*136]"""
                tbl = p1.tile([128, 2 * 128], F32, tag=f"tbl{z}")
                tblb = tbl[:].bitcast(BF16)
                ps3 = ps[:, 0:wn * 136].rearrange("p (w c) -> p w c", c=136)
                tb3 = tblb[:, 0:wn * 256].rearrange("p (w c) -> p w c", c=256)
                tf3 = tbl[:, 0:wn * 128].rearrange("p (w c) -> p w c", c=128)
                # alternate the big xm cast between engines
                if (w // 2) % 2 == 0:
                    nc.scalar.copy(tb3[:, :, 0:128], ps3[:, :, 0:128])
                else:
                    nc.vector.tensor_copy(tb3[:, :, 0:128], ps3[:, :, 0:128])
                nc.vector.tensor_copy(tf3[:, :, 64:68], ps3[:, :, 128:132])
                nc.vector.tensor_copy(tw[:, w:w + wn, z * 4:z * 4 + 4],
                                      ps3[:, :, 132:136])
                dst = st["ag_in"][:].rearrange(
                    "(p w) c -> p w c", p=128)[:, w:w + wn, :]
                nc.sync.dma_start(dst, tf3)

            for z, st in enumerate(sets):
                for w in range(0, NW, 2):
                    wn = min(2, NW - w)
                    ps = psA.tile([128, 280], F32, tag="p1ps")
                    for j in range(wn):
                        nc.tensor.matmul(
                            ps[:, j * 136:j * 136 + 136],
                            lhsT=xall[:, (w + j) * 128:(w + j + 1) * 128],
                            rhs=wall[:, z * 136:z * 136 + 136],
                            start=True, stop=True)
                    if z == 1:
                        psk = psT.tile([128, 512], F32, tag="sTt")
                        for j in range(wn):
                            nc.tensor.matmul(
                                psk[:, j * 128:j * 128 + 128],
                                lhsT=xall[:, (w + j) * 128:(w + j + 1) * 128],
                                rhs=wall[:, 272:400], start=True, stop=True)
                        nc.scalar.activation(
                            out_acc[:, w:w + wn, :],
                            psk[:, 0:wn * 128].rearrange(
                                "p (w c) -> p w c", c=128),
                            ACTF.Copy, scale=eps_skip)
                    build_table_pair(st, ps, w, wn, z)
                emit_ag(st)

            # ---------------- edge phase ----------------
            for z, st in enumerate(sets):
                zn = st["zn"]
                st["idx_sb"] = constp.tile([128, st["T"] * 8], I16,
                                           tag=f"idxsb{zn}", name=f"idxsb{zn}")
                nc.sync.dma_start(st["idx_sb"][:], st["idx"][:])
                st["trel_f"] = constp.tile([128, st["T"]], F32,
                                           tag=f"trelf{zn}", name=f"trelf{zn}")
                nc.sync.dma_start(st["trel_f"][:], st["trel"][:])
                groups, calls = st["groups"], st["calls"]
                trel_f, idx_sb = st["trel_f"], st["idx_sb"]

                # tile index -> (gather ring tile, position-in-call)
                tile_loc = {}
                call_of = {}
                for ci, (cc, t0, nt) in enumerate(calls):
                    g = gatp.tile([128, CALL_TILES * EW], F32, tag="gring")
                    dst = g[:, 0:nt * EW].rearrange("p (t e) -> p t e", e=EW)
                    nidx = nt * 128
                    _dma_gather_slim(
                        nc.gpsimd, dst,
                        st["ag_out"][cc * 4 * SH:(cc + 1) * 4 * SH, 0:EW],
                        idx_sb[:, t0 * 8:t0 * 8 + nt * 8], nidx, nidx, EW,
                        128, queue_num=ci % 4)
                    for j in range(nt):
                        tile_loc[t0 + j] = (g, j)
                        call_of[t0 + j] = ci

                # per-call batched alpha/e_att/scale state
                call_state = {}

                wg_of = {}
                for gi, (cc, wg, cap, t0) in enumerate(groups):
                    for j in range(cap):
                        wg_of[t0 + j] = wg

                def process_call(ci):
                    """S compare, S^T (PE broadcast + 4x compare), t-expand,
                    alpha, exp, scale — batched per call. The S^T/t-expand
                    prep depends only on static inputs, so it runs ahead of
                    the gather-dependent alpha chain."""
                    cc, t0, nt = calls[ci]
                    g = tile_loc[t0][0]
                    # ---- static prep (no gather dependency): runs ahead ----
                    with tc.high_priority(offset=2000):
                        Sc = workp.tile([128, CALL_TILES * 128], BF16, tag="S",
                                        name=f"S_{z}_{ci}", bufs=6)
                        iota3 = iota_bf[:].rearrange("p (o e) -> p o e", o=1)
                        trel3 = trel_f[:, t0:t0 + nt].rearrange(
                            "p (t o) -> p t o", o=1)
                        i3, t3 = bass.broadcast_tensor_aps(iota3, trel3)
                        nc.vector.tensor_tensor(
                            Sc[:, 0:nt * 128].rearrange(
                                "p (t e) -> p t e", e=128),
                            i3, t3, AL.is_equal)
                        # t-value rows -> all partitions via PE broadcast,
                        # staged to SBUF by Scalar so the S^T compare runs
                        # at 4x on DVE
                        stg = smallp.tile([1, CALL_TILES * 128], BF16,
                                          tag="trowstg", name=f"stg_{z}_{ci}")
                        nc.sync.dma_start(
                            stg[:, 0:nt * 128],
                            st["trow"][0:1, t0 * 128:(t0 + nt) * 128])
                        pbc_sb = workp.tile([128, CALL_TILES * 128], BF16,
                                            tag="pbcs", name=f"pbcs_{z}_{ci}",
                                            bufs=6)
                        for mm0 in range(0, nt * 128, 512):
                            mm1 = min(mm0 + 512, nt * 128)
                            pbc = psT.tile([128, 512], F32, tag="sTt",
                                           name=f"pbc_{z}_{ci}_{mm0}")
                            nc.tensor.matmul(pbc[:, 0:mm1 - mm0],
                                             lhsT=ones_row[:],
                                             rhs=stg[0:1, mm0:mm1],
                                             start=True, stop=True)
                            nc.scalar.copy(pbc_sb[:, mm0:mm1],
                                           pbc[:, 0:mm1 - mm0])
                        STc = workp.tile([128, CALL_TILES * 128], BF16,
                                         tag="STc", name=f"STc_{z}_{ci}",
                                         bufs=6)
                        nc.vector.tensor_scalar(STc[:, 0:nt * 128],
                                                pbc_sb[:, 0:nt * 128],
                                                iota_col[:], None, AL.is_equal)
                        # t-expand per tile (PE) into one per-call bank
                        pte = psA.tile([128, CALL_TILES * 4], F32, tag="p1ps",
                                       name=f"pte_{z}_{ci}")
                        for j in range(nt):
                            wg = wg_of[t0 + j]
                            nc.tensor.matmul(
                                pte[:, j * 4:j * 4 + 4],
                                lhsT=STc[:, j * 128:(j + 1) * 128],
                                rhs=tw[:, wg, z * 4:z * 4 + 4],
                                start=True, stop=True)
                    # alpha = s + t ; lrelu ; exp -> B ; scale
                    al = smallp.tile([128, CALL_TILES * 4], F32, tag="al",
                                     name=f"al_{z}_{ci}")
                    alv = al[:, 0:nt * 4]
                    s_ap = g[:, 64:68]
                    s_ap3 = bass.AP(s_ap.tensor, s_ap.offset,
                                    [s_ap.ap[0], [EW, nt], [1, 4]])
                    nc.vector.tensor_tensor(
                        alv.rearrange("p (t f) -> p t f", f=4), s_ap3,
                        pte[:, 0:nt * 4].rearrange("p (t f) -> p t f", f=4),
                        AL.add)
                    nc.vector.scalar_tensor_tensor(alv, alv, 0.01, alv,
                                                   AL.mult, AL.max)
                    B = workp.tile([128, CALL_TILES, 132], BF16, tag="B",
                                   name=f"B_{z}_{ci}", bufs=6)
                    nc.scalar.activation(
                        B[:, 0:nt, 128:132],
                        alv.rearrange("p (t f) -> p t f", f=4), ACTF.Exp)
                    if EB_DENSE:
                        # dense e_att expansion on Scalar (stride-0 src read)
                        # so the message multiply runs at 2x on DVE
                        Eb = workp.tile([128, CALL_TILES * 128], BF16,
                                        tag="Eb", name=f"Eb_{z}_{ci}", bufs=6)
                        b_sl = B[:, 0:nt, 128:132]
                        eb = bass.AP(b_sl.tensor, b_sl.offset,
                                     [*b_sl.ap, [0, 32]])
                        nc.scalar.copy(
                            Eb[:, 0:nt * 128].rearrange(
                                "p (t h c) -> p t h c", h=4, c=32), eb)
                        gbf = g[:].bitcast(BF16)
                        mb = bass.AP(gbf.tensor, gbf.offset,
                                     [gbf.ap[0], [2 * EW, nt], [1, 128]])
                        nc.vector.tensor_tensor(
                            B[:, 0:nt, 0:128], mb,
                            Eb[:, 0:nt * 128].rearrange(
                                "p (t c) -> p t c", c=128),
                            AL.mult)
                    else:
                        gbf = g[:].bitcast(BF16)
                        mb = bass.AP(gbf.tensor, gbf.offset,
                                     [gbf.ap[0], [2 * EW, nt], [32, 4], [1, 32]])
                        b_sl = B[:, 0:nt, 128:132]
                        eb = bass.AP(b_sl.tensor, b_sl.offset,
                                     [*b_sl.ap, [0, 32]])
                        nc.vector.tensor_tensor(
                            B[:, 0:nt, 0:128].rearrange(
                                "p t (h c) -> p t h c", h=4), mb, eb, AL.mult)
                    call_state[ci] = (Sc, B)

                # scatter matmuls in stream order, windows accumulate in PSUM
                flushed = set()
                for gi, (cc, wg, cap, t0) in enumerate(groups):
                    pw = psW.tile([128, 132], F32, tag="pw",
                                  name=f"pw_{z}_{cc}_{wg}")
                    for j in range(cap):
                        ci = call_of[t0 + j]
                        if ci not in call_state:
                            process_call(ci)
                            # retire old call states (ring depth)
                            for old in [k for k in call_state
                                        if k < ci - RING + 1]:
                                del call_state[old]
                        Sc, B = call_state[ci]
                        _, jj = tile_loc[t0 + j]
                        nc.tensor.matmul(pw[:],
                                         lhsT=Sc[:, jj * 128:(jj + 1) * 128],
                                         rhs=B[:, jj, :],
                                         start=(j == 0), stop=(j == cap - 1))
                    if wg not in flushed:
                        nc.scalar.copy(wacc[:, wg, :], pw[:])
                        flushed.add(wg)
                    else:
                        nc.vector.tensor_add(wacc[:, wg, :],
                                             wacc[:, wg, :], pw[:])

                # ---- batched epilogue for this set ----
                # rec = 1/(den+eps); out_acc += num * rec (head-broadcast)
                den = smallp.tile([128, NW * 4], F32, tag="den",
                                  name=f"den_{z}")
                nc.vector.tensor_single_scalar(
                    den[:].rearrange("p (w f) -> p w f", f=4),
                    wacc[:, :, 128:132], 1e-16, AL.add)
                rec = smallp.tile([128, NW * 4], F32, tag="rec",
                                  name=f"rec_{z}")
                nc.vector.reciprocal(rec[:], den[:])
                num4 = wacc[:, :, 0:128].rearrange(
                    "p w (h c) -> p w h c", h=4)
                rec4 = rec[:].rearrange("p (w h o) -> p w h o", h=4, o=1)
                n4, r4 = bass.broadcast_tensor_aps(num4, rec4)
                tmp = winp.tile([128, NW, 128], F32, tag="tmpn",
                                name=f"tmp_{z}")
                nc.vector.tensor_tensor(
                    tmp[:].rearrange("p w (h c) -> p w h c", h=4),
                    n4, r4, AL.mult)
                nc.vector.tensor_add(out_acc[:], out_acc[:], tmp[:])

            # ---- final relu + single p-major store ----
            nc.scalar.activation(out_acc[:], out_acc[:], ACTF.Relu)
            dst = out_ext[:].rearrange("(p w) c -> p w c", p=128)
            nc.sync.dma_start(dst, out_acc[:])

    nc.compile()
    return nc


# ----------------------------------------------------------------------------
# entry point
# ----------------------------------------------------------------------------

def _prepare(x, W_low, a_src_low, a_dst_low, W_up, a_src_up, a_dst_up, W_skip,
             lower_tgt, lower_src, upper_tgt, upper_src):
    n, inch = x.shape
    local = n // NCORES
    assert local == 6250 and inch == 128
    assert 4 * SH <= 32767, "int16 gather index overflow"

    lower_tgt = np.asarray(lower_tgt); lower_src = np.asarray(lower_src)
    upper_tgt = np.asarray(upper_tgt); upper_src = np.asarray(upper_src)
    perms = _make_perms(lower_tgt, lower_src, upper_tgt, upper_src, local)
    capsL, groupsL, callsL, TL, coresL = _preprocess(
        lower_tgt, lower_src, local, perms)
    capsU, groupsU, callsU, TU, coresU = _preprocess(
        upper_tgt, upper_src, local, perms)

    meta = dict(eps_skip=1.0 + 1e-6,
                caps=[capsL, capsU], groups=[groupsL, groupsU],
                calls=[callsL, callsU], T=[TL, TU], perms=perms)

    import ml_dtypes
    W_low = np.asarray(W_low, np.float32)
    W_up = np.asarray(W_up, np.float32)
    W_skip = np.asarray(W_skip, np.float32)
    wall = np.concatenate([
        W_low, W_low @ _block_diag_a(np.asarray(a_src_low)),
        W_low @ _block_diag_a(np.asarray(a_dst_low)),
        W_up, W_up @ _block_diag_a(np.asarray(a_src_up)),
        W_up @ _block_diag_a(np.asarray(a_dst_up)),
        W_skip], axis=1).astype(ml_dtypes.bfloat16)

    x = np.asarray(x, np.float32)
    in_maps = []
    for k in range(NCORES):
        xk = np.zeros((SH, inch), np.float32)
        xk[perms[k]] = x[k * local:(k + 1) * local]
        in_maps.append({
            "xT": np.ascontiguousarray(xk.T).astype(ml_dtypes.bfloat16),
            "Wall": wall,
            "idxL": coresL[k]["idx"], "trelL": coresL[k]["trel"],
            "trowL": coresL[k]["trow"],
            "idxU": coresU[k]["idx"], "trelU": coresU[k]["trel"],
            "trowU": coresU[k]["trow"],
        })
    return meta, in_maps, local


def kernel(x, W_low, a_src_low, a_dst_low, W_up, a_src_up, a_dst_up, W_skip,
           lower_tgt, lower_src, upper_tgt, upper_src):
    from concourse.bass_utils import run_bass_kernel_spmd

    meta, in_maps, local = _prepare(
        x, W_low, a_src_low, a_dst_low, W_up, a_src_up, a_dst_up, W_skip,
        lower_tgt, lower_src, upper_tgt, upper_src)
    nc = _build(meta)

    res = run_bass_kernel_spmd(nc, in_maps, list(range(NCORES)), trace=TRACE)
    LAST_RESULT["exec_time_ns"] = res.exec_time_ns
    LAST_RESULT["res"] = res

    n = np.asarray(x).shape[0]
    perms = meta["perms"]
    out = np.empty((n, 128), np.float32)
    for k in range(NCORES):
        ok = np.asarray(res.results[k]["out"])
        # out rows are p-major: row = p*NW + w for slot (w, p)
        sl = perms[k]
        rows = (sl % WIN) * NW + sl // WIN
        out[k * local:(k + 1) * local] = ok[rows]
    return out


# revision 54
# speedup vs baseline: 1.4642x; 1.1163x over previous
"""CANLayer (cell attention) distributed Bass kernel for 8 TRN2 NeuronCores.

Strategy (graph/data parallel by destination cell, per sharding hint):
 - core k owns target nodes [k*LOCAL, (k+1)*LOCAL)
 - the per-node table row id is partition-major (rho = p*NW + w) so the
   phase-1 table store is a contiguous per-partition DMA.
 - one AllGather per set (chunking proved counterproductive: each collective
   pays a ~30us peer-barrier); AG-L fires right after the L table is stored,
   AG-U after the U table, both before any gathers on the in-order gpsimd
   queue (their inputs are ready by then, so they don't stall gather
   dispatch).
 - edge phase per set: edges sorted by (src-core-half, target-window);
   8-tile dma_gather calls read only the 272B payload of each 512B-strided
   table row on 4 SWDGE queues with a ring of gather buffers; per 128-edge
   tile build one-hot S [e,t] via DVE compare vs iota; S^T via ones-row PE
   broadcast + DVE compare; alpha -> lrelu -> exp; scatter =
   matmul(lhsT=S, rhs=[M*e_att | e_att]) accumulated in PSUM per 128-target
   window; per-set batched epilogue normalizes by the denominator columns;
   final relu + single p-major output store.
"""
import sys

if "/opt/trn_rl_repo" not in sys.path:
    sys.path.insert(0, "/opt/trn_rl_repo")

import numpy as np

TRACE = False          # test.py sets True to get exec_time_ns + perfetto
LAST_RESULT = {}       # test.py reads exec_time_ns etc. from here

NCORES = 8
WIN = 128              # targets per PSUM window
NW = 50                # windows per core
SH = NW * WIN          # padded rows per core (6400)
CALL_TILES = 8         # max 128-edge tiles per dma_gather call
TCALLS = 1             # main calls covered by one t-value gather
RING = 8               # gather ring depth (pool bufs)
COLL_BYPASS = False    # diagnostic: replace AllGather with local shard copy
EW = 68                # f32 cols gathered per edge (272B payload)
EB_DENSE = True        # expand e_att densely on Scalar for a 2x DVE multiply


def _dma_gather_slim(gp, out_ap, in_ap, idxs_ap, num_idxs, num_idxs_reg,
                     elem_size, elem_step, queue_num=0):
    """nc.gpsimd.dma_gather with the elem_size%256B assert relaxed.

    The SWDGE ISA encodes the row stride in 256B units (elem_step must be a
    256B multiple) but the per-descriptor read size is a plain byte count;
    reading a 272B payload out of 512B-strided rows is legal at the
    descriptor level. Only HBM-source, transpose=False is supported here.
    """
    import concourse.mybir as mybir
    from concourse import ap_utils
    from concourse.bass_primitives import MemorySpace
    from concourse._compat import round_up_to_multiple, exact_div

    gp._assert_queue_num(queue_num)
    assert idxs_ap.dtype == mybir.dt.int16
    assert in_ap.dtype == out_ap.dtype
    assert in_ap.space == MemorySpace.DRAM
    assert idxs_ap.space == MemorySpace.SBUF
    assert out_ap.space == MemorySpace.SBUF
    assert ap_utils.ap_is_contiguous(out_ap.ap[1:])
    assert ap_utils.ap_is_contiguous(idxs_ap.ap[1:])
    assert in_ap.ap[-1][1] == out_ap.ap[-1][1] == elem_size
    assert out_ap.ap[0][1] * out_ap.ap[1][1] == round_up_to_multiple(num_idxs, 128)
    assert in_ap.ap[0][0] == elem_step
    stride_bytes = elem_step * mybir.dt.size(in_ap.dtype)
    stride_bytes_256 = exact_div(stride_bytes, 256)
    assert stride_bytes_256 < 256

    _in_ap = gp.lower_ap_dma(in_ap, for_custom_bir_dma=True)
    _idxs_ap = gp.lower_ap(idxs_ap)
    _out_ap = gp.lower_ap(out_ap)
    return gp.add_instruction(
        mybir.InstDMAGatherAnt(
            name=gp.bass.get_next_instruction_name(),
            ins=[*_in_ap, _idxs_ap,
                 gp.lower_val_access(gp.to_reg(num_idxs_reg))],
            outs=[_out_ap],
            transpose=False,
            num_idxs=num_idxs,
            elem_size=elem_size,
            stride_bytes_256=stride_bytes_256,
            gen_mode=0,
            single_packet=True,
            queue_num=queue_num,
            sbuf_tokens_per_rank=0,
            sbuf_free_dim_per_rank=0,
            sbuf_free_dim_pad_per_rank=0,
            sbuf_byte_offset=0,
        )
    )


def _binpack_windows(degs, local, budget):
    """degs: [n_kinds, local] per-node edge counts. Returns flat slot
    (w*WIN + pos) per node. Greedy first-fit-decreasing."""
    budgets = np.full((NW, degs.shape[0]), budget, np.int64)
    budgets[max(0, NW - 6):] = 6 * WIN
    counts = np.zeros((NW, degs.shape[0]), np.int64)
    nslots = np.zeros(NW, np.int64)
    win_of = np.full(local, -1, np.int64)
    order = np.argsort(-degs.sum(0), kind="stable")
    cap_slots = np.full(NW, local // NW, np.int64)
    cap_slots[:local - (local // NW) * NW] += 1
    for j in order:
        d = degs[:, j]
        over = np.maximum(counts + d[None, :] - budgets, 0).sum(1)
        slack = (budgets - counts - d[None, :]).sum(1)
        cost = over * 100000 - slack
        cost[nslots >= cap_slots] = 1 << 60
        w = int(np.argmin(cost))
        counts[w] += d
        win_of[j] = w
        nslots[w] += 1
    slot = np.empty(local, np.int64)
    used = np.zeros(NW, np.int64)
    for j in range(local):
        w = win_of[j]
        slot[j] = w * WIN + used[w]
        used[w] += 1
    return slot


def _make_perms(lower_tgt, lower_src, upper_tgt, upper_src, local):
    """perm[k][t] = global slot (window-major) of node t."""
    perms = []
    for k in range(NCORES):
        degs = []
        for tg, sr in ((lower_tgt, lower_src), (upper_tgt, upper_src)):
            m = (tg // local) == k
            tl = (tg[m] - k * local).astype(np.int64)
            ch = (sr[m].astype(np.int64) // local) // 4   # src core half
            for h in (0, 1):
                degs.append(np.bincount(tl[ch == h], minlength=local))
        perms.append(_binpack_windows(np.stack(degs), local, 505))
    return perms


def _rho(slot):
    """table row id: partition-major (p*NW + w)."""
    w = slot // WIN
    p = slot % WIN
    return (p * NW + w).astype(np.int64)


# ----------------------------------------------------------------------------
# host-side index preprocessing (pure layout/index manipulation)
# ----------------------------------------------------------------------------

def _preprocess(tgt, src, local, perms):
    """Shard one edge set by destination core; sort by (src-half, window).

    Static structure (caps, groups, calls) is shared across cores (maxed)
    as required for SPMD; per-core arrays carry indices + relative targets.
    """
    per_core = []
    for k in range(NCORES):
        m = (tgt // local) == k
        s = src[m].astype(np.int64)
        tl = perms[k][(tgt[m] - k * local).astype(np.int64)]
        c = (s // local) // 4        # src core half
        w = tl // WIN
        order = np.lexsort((tl, w, c))
        per_core.append((s[order], tl[order], w[order], c[order]))

    caps = np.zeros((2, NW), np.int64)
    for k in range(NCORES):
        s, tl, w, c = per_core[k]
        for cc in range(2):
            cnt = np.bincount(w[c == cc], minlength=NW)
            caps[cc] = np.maximum(caps[cc], (cnt + WIN - 1) // WIN)

    # groups in stream order; calls are CALL_TILES-sized slices of each
    # bucket run (groups may span calls)
    groups = []          # (cc, w, cap, tile_start)
    calls = []           # (cc, tile_start, n_tiles)
    t_idx = 0
    for cc in range(2):
        run_start = t_idx
        for w in range(NW):
            cap = int(caps[cc][w])
            if cap == 0:
                continue
            groups.append((cc, w, cap, t_idx))
            t_idx += cap
        for c0 in range(run_start, t_idx, CALL_TILES):
            calls.append((cc, c0, min(CALL_TILES, t_idx - c0)))
    T = t_idx

    cores = []
    for k in range(NCORES):
        s, tl, w, c = per_core[k]
        slots = T * 128
        src16 = np.zeros(slots, np.int16)          # pad -> idx 0 (valid row)
        trel = np.full(slots, -1.0, np.float32)    # pad -> -1 (no S match)
        for (cc, wg, cap, t0) in groups:
            sel = (c == cc) & (w == wg)
            n = int(sel.sum())
            off = t0 * 128
            ss = s[sel]
            sloc = np.empty(len(ss), np.int64)
            for ks in range(NCORES):
                mm = (ss // local) == ks
                sloc[mm] = perms[ks][ss[mm] % local]
            src16[off:off + n] = (((ss // local) % 4) * SH + _rho(sloc)
                                  ).astype(np.int16)
            trel[off:off + n] = (tl[sel] - wg * WIN).astype(np.float32)
        # local target rows for the t-value gather (pads -> row 0)
        tl16 = np.zeros(slots, np.int16)
        for (cc, wg, cap, t0) in groups:
            sel = (c == cc) & (w == wg)
            n = int(sel.sum())
            off = t0 * 128
            tl16[off:off + n] = _rho(tl[sel]).astype(np.int16)
        ii = np.arange(slots)
        idxarr = np.zeros((128, T * 8), np.int16)
        idxtarr = np.zeros((128, T * 8), np.int16)
        for g8 in range(8):
            idxarr[g8 * 16 + ii % 16, ii // 16] = src16
            idxtarr[g8 * 16 + ii % 16, ii // 16] = tl16
        trelarr = np.full((128, T), -1.0, np.float32)
        trelarr[ii % 128, ii // 128] = trel
        cores.append({"idx": idxarr, "idxt": idxtarr, "trel": trelarr})
    # every window must receive at least one scatter flush, else the batched
    # epilogue would read garbage accumulators for it
    assert {w for (_, w, _, _) in groups} == set(range(NW))
    return caps, groups, calls, T, cores


def _block_diag_a(a):  # [H, C] -> [H*C, H] block diagonal (layout only)
    h, c = a.shape
    out = np.zeros((h * c, h), np.float32)
    for i in range(h):
        out[i * c:(i + 1) * c, i] = a[i]
    return out


# ----------------------------------------------------------------------------
# device kernel builder
# ----------------------------------------------------------------------------

def _build(meta):
    import concourse.bass as bass
    import concourse.bacc as bacc
    import concourse.mybir as mybir
    import concourse.tile as tile

    F32 = mybir.dt.float32
    BF16 = mybir.dt.bfloat16
    I16 = mybir.dt.int16
    I32 = mybir.dt.int32
    AL = mybir.AluOpType
    ACTF = mybir.ActivationFunctionType

    eps_skip = meta["eps_skip"]

    nc = bacc.Bacc("TRN2", target_bir_lowering=False, debug=False,
                   num_devices=NCORES, num_swdge_queues=4)

    xT = nc.dram_tensor("xT", [128, SH], BF16, kind="ExternalInput")
    # wall = [Wl | Wl@As_l | Wl@Ad_l | Wu | Wu@As_u | Wu@Ad_u | Wskip]
    Wall = nc.dram_tensor("Wall", [128, 400], BF16, kind="ExternalInput")
    out_ext = nc.dram_tensor("out", [SH, 128], F32, kind="ExternalOutput")

    sets = []
    for z, zn in enumerate("LU"):
        TZ = meta["T"][z]
        sets.append(dict(
            z=z, zn=zn,
            idx=nc.dram_tensor(f"idx{zn}", [128, TZ * 8], I16, kind="ExternalInput"),
            idxt=nc.dram_tensor(f"idxt{zn}", [128, TZ * 8], I16, kind="ExternalInput"),
            trel=nc.dram_tensor(f"trel{zn}", [128, TZ], F32, kind="ExternalInput"),
            ag_in=nc.dram_tensor(f"agin{zn}", [SH, 128], F32),
            ag_out=nc.dram_tensor(f"agout{zn}", [NCORES * SH, 128], F32,
                                  addr_space="Shared"),
            caps=meta["caps"][z], groups=meta["groups"][z],
            calls=meta["calls"][z], T=TZ,
        ))

    rg = [list(range(NCORES))]

    with tile.TileContext(nc) as tc:
        with (
            tc.tile_pool(name="const", bufs=1) as constp,
            tc.tile_pool(name="p1", bufs=3) as p1,
            tc.tile_pool(name="gat", bufs=RING) as gatp,
            tc.tile_pool(name="work", bufs=6) as workp,
            tc.tile_pool(name="small", bufs=4) as smallp,
            tc.tile_pool(name="winb", bufs=1) as winp,
            tc.tile_pool(name="psA", bufs=2, space="PSUM") as psA,
            tc.tile_pool(name="psT", bufs=4, space="PSUM") as psT,
            tc.tile_pool(name="psW", bufs=2, space="PSUM") as psW,
        ):
            # ---------------- constants ----------------
            # xall + weights first: they gate phase 1 (idx/trel loads are
            # emitted in the edge section so they don't queue ahead)
            xall = winp.tile([128, NW * 128], BF16)
            nc.sync.dma_start(xall[:], xT[:])
            wall = constp.tile([128, 400], BF16)
            nc.sync.dma_start(wall[:], Wall[:])

            iota_i = constp.tile([128, 128], I32)
            nc.gpsimd.iota(iota_i[:], [[1, 128]], base=0, channel_multiplier=0)
            iota_bf = constp.tile([128, 128], BF16)
            nc.vector.tensor_copy(iota_bf[:], iota_i[:])


            # ---------------- persistent buffers ----------------
            out_acc = winp.tile([128, NW, 128], F32)
            wacc = winp.tile([128, NW, 132], F32)      # reused across sets

            def emit_ag(st):
                if COLL_BYPASS:
                    nc.sync.dma_start(st["ag_out"][0:SH, :], st["ag_in"][:])
                else:
                    nc.gpsimd.collective_compute(
                        "AllGather", AL.bypass, replica_groups=rg,
                        ins=[st["ag_in"][:].opt()],
                        outs=[st["ag_out"][:].opt()])

            # ---------------- phase 1 ----------------
            # table rows are partition-major: row rho = p*NW + w
            # -> contiguous per-partition store
            def build_table_pair(st, ps, w, wn, z):
                """table rows for windows [w, w+wn) from psum [128, wn*136]"""
                tbl = p1.tile([128, 2 * 128], F32, tag=f"tbl{z}")
                tblb = tbl[:].bitcast(BF16)
                ps3 = ps[:, 0:wn * 136].rearrange("p (w c) -> p w c", c=136)
                tb3 = tblb[:, 0:wn * 256].rearrange("p (w c) -> p w c", c=256)
                tf3 = tbl[:, 0:wn * 128].rearrange("p (w c) -> p w c", c=128)
                # alternate the big xm cast between engines
                if (w // 2) % 2 == 0:
                    nc.scalar.copy(tb3[:, :, 0:128], ps3[:, :, 0:128])
                else:
                    nc.vector.tensor_copy(tb3[:, :, 0:128], ps3[:, :, 0:128])
                nc.vector.tensor_copy(tf3[:, :, 64:72], ps3[:, :, 128:136])
                dst = st["ag_in"][:].rearrange(
                    "(p w) c -> p w c", p=128)[:, w:w + wn, :]
                nc.sync.dma_start(dst, tf3)

            for z, st in enumerate(sets):
                for w in range(0, NW, 2):
                    wn = min(2, NW - w)
                    ps = psA.tile([128, 280], F32, tag="p1ps")
                    for j in range(wn):
                        nc.tensor.matmul(
                            ps[:, j * 136:j * 136 + 136],
                            lhsT=xall[:, (w + j) * 128:(w + j + 1) * 128],
                            rhs=wall[:, z * 136:z * 136 + 136],
                            start=True, stop=True)
                    if z == 1:
                        psk = psT.tile([128, 512], F32, tag="sTt")
                        for j in range(wn):
                            nc.tensor.matmul(
                                psk[:, j * 128:j * 128 + 128],
                                lhsT=xall[:, (w + j) * 128:(w + j + 1) * 128],
                                rhs=wall[:, 272:400], start=True, stop=True)
                        nc.scalar.activation(
                            out_acc[:, w:w + wn, :],
                            psk[:, 0:wn * 128].rearrange(
                                "p (w c) -> p w c", c=128),
                            ACTF.Copy, scale=eps_skip)
                    build_table_pair(st, ps, w, wn, z)
                emit_ag(st)

            # ---------------- edge phase ----------------
            for z, st in enumerate(sets):
                zn = st["zn"]
                st["idx_sb"] = constp.tile([128, st["T"] * 8], I16,
                                           tag=f"idxsb{zn}", name=f"idxsb{zn}")
                nc.sync.dma_start(st["idx_sb"][:], st["idx"][:])
                st["idxt_sb"] = constp.tile([128, st["T"] * 8], I16,
                                            tag=f"idxtsb{zn}", name=f"idxtsb{zn}")
                nc.sync.dma_start(st["idxt_sb"][:], st["idxt"][:])
                st["trel_f"] = constp.tile([128, st["T"]], F32,
                                           tag=f"trelf{zn}", name=f"trelf{zn}")
                nc.sync.dma_start(st["trel_f"][:], st["trel"][:])
                groups, calls = st["groups"], st["calls"]
                trel_f, idx_sb = st["trel_f"], st["idx_sb"]

                # tile index -> (gather ring tile, position-in-call)
                tile_loc = {}
                call_of = {}
                tg_loc = {}       # call index -> (t-gather tile, tile offset)
                for ci, (cc, t0, nt) in enumerate(calls):
                    if ci % TCALLS == 0:
                        # t-value gather covering the next TCALLS calls,
                        # interleaved with the main gathers so ring-buffer
                        # waits can't deadlock the in-order gpsimd queue
                        grp = calls[ci:ci + TCALLS]
                        gt0 = grp[0][1]
                        ntt = grp[-1][1] + grp[-1][2] - gt0
                        tg = gatp.tile([128, TCALLS * CALL_TILES * 4], F32,
                                       tag="tgring", name=f"tg_{z}_{ci}",
                                       bufs=4)
                        dstt = tg[:, 0:ntt * 4].rearrange(
                            "p (t e) -> p t e", e=4)
                        nidx = ntt * 128
                        _dma_gather_slim(
                            nc.gpsimd, dstt, st["ag_in"][:, 68:72],
                            st["idxt_sb"][:, gt0 * 8:(gt0 + ntt) * 8],
                            nidx, nidx, 4, 128, queue_num=ci % 4)
                        for cj in range(ci, min(ci + TCALLS, len(calls))):
                            tg_loc[cj] = (tg, calls[cj][1] - gt0)
                    g = gatp.tile([128, CALL_TILES * EW], F32, tag="gring")
                    dst = g[:, 0:nt * EW].rearrange("p (t e) -> p t e", e=EW)
                    nidx = nt * 128
                    _dma_gather_slim(
                        nc.gpsimd, dst,
                        st["ag_out"][cc * 4 * SH:(cc + 1) * 4 * SH, 0:EW],
                        idx_sb[:, t0 * 8:t0 * 8 + nt * 8], nidx, nidx, EW,
                        128, queue_num=ci % 4)
                    for j in range(nt):
                        tile_loc[t0 + j] = (g, j)
                        call_of[t0 + j] = ci

                # per-call batched alpha/e_att/scale state
                call_state = {}

                wg_of = {}
                for gi, (cc, wg, cap, t0) in enumerate(groups):
                    for j in range(cap):
                        wg_of[t0 + j] = wg

                def process_call(ci):
                    """S compare, alpha from gathered s and t, exp, scale."""
                    cc, t0, nt = calls[ci]
                    g = tile_loc[t0][0]
                    tg, tgoff = tg_loc[ci]
                    Sc = workp.tile([128, CALL_TILES * 128], BF16, tag="S",
                                    name=f"S_{z}_{ci}", bufs=8)
                    iota3 = iota_bf[:].rearrange("p (o e) -> p o e", o=1)
                    trel3 = trel_f[:, t0:t0 + nt].rearrange(
                        "p (t o) -> p t o", o=1)
                    i3, t3 = bass.broadcast_tensor_aps(iota3, trel3)
                    nc.vector.tensor_tensor(
                        Sc[:, 0:nt * 128].rearrange("p (t e) -> p t e", e=128),
                        i3, t3, AL.is_equal)
                    # alpha = s + t ; lrelu ; exp -> B den cols
                    al = smallp.tile([128, CALL_TILES * 4], F32, tag="al",
                                     name=f"al_{z}_{ci}")
                    alv = al[:, 0:nt * 4]
                    s_ap = g[:, 64:68]
                    s_ap3 = bass.AP(s_ap.tensor, s_ap.offset,
                                    [s_ap.ap[0], [EW, nt], [1, 4]])
                    nc.vector.tensor_tensor(
                        alv.rearrange("p (t f) -> p t f", f=4), s_ap3,
                        tg[:, tgoff * 4:(tgoff + nt) * 4].rearrange(
                            "p (t f) -> p t f", f=4),
                        AL.add)
                    nc.vector.scalar_tensor_tensor(alv, alv, 0.01, alv,
                                                   AL.mult, AL.max)
                    B = workp.tile([128, CALL_TILES, 132], BF16, tag="B",
                                   name=f"B_{z}_{ci}", bufs=8)
                    nc.scalar.activation(
                        B[:, 0:nt, 128:132],
                        alv.rearrange("p (t f) -> p t f", f=4), ACTF.Exp)
                    if EB_DENSE:
                        # dense e_att expansion on Scalar (stride-0 src read)
                        # so the message multiply runs at 2x on DVE
                        Eb = workp.tile([128, CALL_TILES * 128], BF16,
                                        tag="Eb", name=f"Eb_{z}_{ci}", bufs=8)
                        b_sl = B[:, 0:nt, 128:132]
                        eb = bass.AP(b_sl.tensor, b_sl.offset,
                                     [*b_sl.ap, [0, 32]])
                        nc.scalar.copy(
                            Eb[:, 0:nt * 128].rearrange(
                                "p (t h c) -> p t h c", h=4, c=32), eb)
                        gbf = g[:].bitcast(BF16)
                        mb = bass.AP(gbf.tensor, gbf.offset,
                                     [gbf.ap[0], [2 * EW, nt], [1, 128]])
                        nc.vector.tensor_tensor(
                            B[:, 0:nt, 0:128], mb,
                            Eb[:, 0:nt * 128].rearrange(
                                "p (t c) -> p t c", c=128),
                            AL.mult)
                    else:
                        gbf = g[:].bitcast(BF16)
                        mb = bass.AP(gbf.tensor, gbf.offset,
                                     [gbf.ap[0], [2 * EW, nt], [32, 4], [1, 32]])
                        b_sl = B[:, 0:nt, 128:132]
                        eb = bass.AP(b_sl.tensor, b_sl.offset,
                                     [*b_sl.ap, [0, 32]])
                        nc.vector.tensor_tensor(
                            B[:, 0:nt, 0:128].rearrange(
                                "p t (h c) -> p t h c", h=4), mb, eb, AL.mult)
                    call_state[ci] = (Sc, B)

                # scatter matmuls in stream order, windows accumulate in PSUM
                flushed = set()
                for gi, (cc, wg, cap, t0) in enumerate(groups):
                    pw = psW.tile([128, 132], F32, tag="pw",
                                  name=f"pw_{z}_{cc}_{wg}")
                    for j in range(cap):
                        ci = call_of[t0 + j]
                        if ci not in call_state:
                            process_call(ci)
                            # retire old call states (ring depth)
                            for old in [k for k in call_state
                                        if k < ci - RING + 1]:
                                del call_state[old]
                        Sc, B = call_state[ci]
                        _, jj = tile_loc[t0 + j]
                        nc.tensor.matmul(pw[:],
                                         lhsT=Sc[:, jj * 128:(jj + 1) * 128],
                                         rhs=B[:, jj, :],
                                         start=(j == 0), stop=(j == cap - 1))
                    if wg not in flushed:
                        nc.scalar.copy(wacc[:, wg, :], pw[:])
                        flushed.add(wg)
                    else:
                        nc.vector.tensor_add(wacc[:, wg, :],
                                             wacc[:, wg, :], pw[:])

                # ---- batched epilogue for this set ----
                # rec = 1/(den+eps); out_acc += num * rec (head-broadcast)
                den = smallp.tile([128, NW * 4], F32, tag="den",
                                  name=f"den_{z}")
                nc.vector.tensor_single_scalar(
                    den[:].rearrange("p (w f) -> p w f", f=4),
                    wacc[:, :, 128:132], 1e-16, AL.add)
                rec = smallp.tile([128, NW * 4], F32, tag="rec",
                                  name=f"rec_{z}")
                nc.vector.reciprocal(rec[:], den[:])
                num4 = wacc[:, :, 0:128].rearrange(
                    "p w (h c) -> p w h c", h=4)
                rec4 = rec[:].rearrange("p (w h o) -> p w h o", h=4, o=1)
                n4, r4 = bass.broadcast_tensor_aps(num4, rec4)
                tmp = winp.tile([128, NW, 128], F32, tag="tmpn",
                                name=f"tmp_{z}")
                nc.vector.tensor_tensor(
                    tmp[:].rearrange("p w (h c) -> p w h c", h=4),
                    n4, r4, AL.mult)
                nc.vector.tensor_add(out_acc[:], out_acc[:], tmp[:])

            # ---- final relu + single p-major store ----
            nc.scalar.activation(out_acc[:], out_acc[:], ACTF.Relu)
            dst = out_ext[:].rearrange("(p w) c -> p w c", p=128)
            nc.sync.dma_start(dst, out_acc[:])

    nc.compile()
    return nc


# ----------------------------------------------------------------------------
# entry point
# ----------------------------------------------------------------------------

def _prepare(x, W_low, a_src_low, a_dst_low, W_up, a_src_up, a_dst_up, W_skip,
             lower_tgt, lower_src, upper_tgt, upper_src):
    n, inch = x.shape
    local = n // NCORES
    assert local == 6250 and inch == 128
    assert 4 * SH <= 32767, "int16 gather index overflow"

    lower_tgt = np.asarray(lower_tgt); lower_src = np.asarray(lower_src)
    upper_tgt = np.asarray(upper_tgt); upper_src = np.asarray(upper_src)
    perms = _make_perms(lower_tgt, lower_src, upper_tgt, upper_src, local)
    capsL, groupsL, callsL, TL, coresL = _preprocess(
        lower_tgt, lower_src, local, perms)
    capsU, groupsU, callsU, TU, coresU = _preprocess(
        upper_tgt, upper_src, local, perms)

    meta = dict(eps_skip=1.0 + 1e-6,
                caps=[capsL, capsU], groups=[groupsL, groupsU],
                calls=[callsL, callsU], T=[TL, TU], perms=perms)

    import ml_dtypes
    W_low = np.asarray(W_low, np.float32)
    W_up = np.asarray(W_up, np.float32)
    W_skip = np.asarray(W_skip, np.float32)
    wall = np.concatenate([
        W_low, W_low @ _block_diag_a(np.asarray(a_src_low)),
        W_low @ _block_diag_a(np.asarray(a_dst_low)),
        W_up, W_up @ _block_diag_a(np.asarray(a_src_up)),
        W_up @ _block_diag_a(np.asarray(a_dst_up)),
        W_skip], axis=1).astype(ml_dtypes.bfloat16)

    x = np.asarray(x, np.float32)
    in_maps = []
    for k in range(NCORES):
        xk = np.zeros((SH, inch), np.float32)
        xk[perms[k]] = x[k * local:(k + 1) * local]
        in_maps.append({
            "xT": np.ascontiguousarray(xk.T).astype(ml_dtypes.bfloat16),
            "Wall": wall,
            "idxL": coresL[k]["idx"], "trelL": coresL[k]["trel"],
            "idxtL": coresL[k]["idxt"],
            "idxU": coresU[k]["idx"], "trelU": coresU[k]["trel"],
            "idxtU": coresU[k]["idxt"],
        })
    return meta, in_maps, local


def kernel(x, W_low, a_src_low, a_dst_low, W_up, a_src_up, a_dst_up, W_skip,
           lower_tgt, lower_src, upper_tgt, upper_src):
    from concourse.bass_utils import run_bass_kernel_spmd

    meta, in_maps, local = _prepare(
        x, W_low, a_src_low, a_dst_low, W_up, a_src_up, a_dst_up, W_skip,
        lower_tgt, lower_src, upper_tgt, upper_src)
    nc = _build(meta)

    res = run_bass_kernel_spmd(nc, in_maps, list(range(NCORES)), trace=TRACE)
    LAST_RESULT["exec_time_ns"] = res.exec_time_ns
    LAST_RESULT["res"] = res

    n = np.asarray(x).shape[0]
    perms = meta["perms"]
    out = np.empty((n, 128), np.float32)
    for k in range(NCORES):
        ok = np.asarray(res.results[k]["out"])
        # out rows are p-major: row = p*NW + w for slot (w, p)
        sl = perms[k]
        rows = (sl % WIN) * NW + sl // WIN
        out[k * local:(k + 1) * local] = ok[rows]
    return out
